# revision 39
# baseline (speedup 1.0000x reference)
"""GRAM model Trainium2 kernel: 8-core SPMD via bass/tile.

Strategy (data-parallel over graphs for the NTN head, vocab-parallel for
the DAG-embedding stage, per the sharding hint):

 - DAG embedding stage sharded over vocab (exact /8 shards): fp16
   transposed gathers (one per group x {anc,leaf}) feed PE matmuls for
   h=tanh(cat@Wl.T+bl); attention logits in [v,l] layout via lhsT=h
   matmuls; softmax per v-tile; global softmax weight sums (13 floats)
   via AllGather + on-chip reduce.
 - all_emb shard is built by re-using the SBUF-resident anc tiles
   (weighted sum over levels with the global sbar weights), then PE
   transposes into [v,h] fp16 lhsT chunks.  No second gather pass.
 - segment-sum + node gather are fused into a count matmul:
   le.T[h,b] = sum_v emb_shard[v,h] * C[v,b], where C is the (vocab-row,
   graph) multiplicity matrix built host-side from the integer index
   tensors.  Each core contracts its own 2304 vocab rows against all
   4096 graphs (streamed from HBM in fp16), and one ReduceScatter(add)
   both sums the partials over cores and leaves each core exactly its
   512-graph block of le/re -- no all_embedding AllGather, no per-node
   gather descriptors.
 - NTN head computed per core on its 512 graph pairs.

Timing: the printed HW exec time is measured differentially -- the same
program is compiled once with the body repeated KREP times on-device and
once plain; (T_rep - T_plain)/(KREP-1) under pipelined dispatch isolates
the on-device execution time from the ~2-70ms host->device dispatch
latency of this environment (an empty kernel measures the same as the
full one in a naive per-call measurement).
"""
import os
import numpy as np

KPH = os.environ.get("KPH", "F")
LAST_RESULT = None
LAST_EXEC_NS = None

H = 128
P16 = 16
B = 4096
T = 262144
V_D, V_P, V_A = 10000, 4000, 4000
LS = [4, 4, 5]
NCORE = 8
BLOC = B // NCORE          # 512 graph pairs per core
VS = [1250, 500, 500]
VPAD = [1280, 512, 512]
NTIL = [10, 4, 4]
MOFF = [0, 10, 14]         # tile-column offsets into the mask array
GCOL = [0, 4, 8]           # sbar column offsets per group
GOFF_SH = [0, 1280, 1792]  # row offset of group inside a rank's shard
SHROWS = 2304              # rows per rank shard (incl pads)
EOFF = [0, 13000, 18200]   # group offsets in emb_cat (23400 rows)
NCHK = SHROWS // 128       # 18 lhsT chunks per core
NBB = B // 512             # 8 graph blocks of 512
DAGROWS = sum(VPAD[g] * LS[g] for g in range(3))   # 9728


def _build_perm():
    perm = np.empty(18000, np.int64)
    v = np.arange(V_D)
    perm[:V_D] = (v // VS[0]) * SHROWS + GOFF_SH[0] + (v % VS[0])
    v = np.arange(V_P)
    perm[V_D:V_D + V_P] = (v // VS[1]) * SHROWS + GOFF_SH[1] + (v % VS[1])
    v = np.arange(V_A)
    perm[V_D + V_P:] = (v // VS[2]) * SHROWS + GOFF_SH[2] + (v % VS[2])
    return perm


def _wrap_idx(a):
    """dma_gather index layout: element i at [i%16, i//16]; replicate to 128 parts."""
    m = a.reshape(-1, 16).T.astype(np.int16)
    return np.ascontiguousarray(np.tile(m, (8, 1)))


def kernel(**inputs):
    import concourse.bacc as bacc
    import concourse.tile as tile
    import concourse.mybir as mybir
    from concourse import bass_isa
    from concourse.bass_utils import run_bass_kernel_spmd

    f32 = mybir.dt.float32
    f16 = mybir.dt.float16
    i16 = mybir.dt.int16
    f8 = mybir.dt.float8e4

    # ---------------- host-side shard prep ----------------
    lx = np.asarray(inputs["left_x"])[:, 0].astype(np.int64)
    rx = np.asarray(inputs["right_x"])[:, 0].astype(np.int64)
    lb = np.asarray(inputs["left_x_batch"]).astype(np.int64)
    rb = np.asarray(inputs["right_x_batch"]).astype(np.int64)

    perm = _build_perm()

    def count_mats(pos, seg):
        """[core, chk, row, bb, col] multiplicity counts."""
        cnt = np.bincount(pos * B + seg, minlength=NCORE * SHROWS * B)
        mx = int(cnt.max())
        assert mx < 2048, "counts exceed fp16 exact-integer range"
        return cnt.astype(np.float16).reshape(NCORE, NCHK, 128, NBB, 512), mx

    cl, mxl = count_mats(perm[lx], lb)
    cr, mxr = count_mats(perm[rx], rb)
    # fp8e4m3 represents integers exactly up to 16; halves the HBM stream
    use_f8 = max(mxl, mxr) <= 16 and os.environ.get("KC8", "1") != "0"
    import ml_dtypes
    cnp = ml_dtypes.float8_e4m3 if use_f8 else np.float16
    # per-core layout: rows (bb, side, 128), cols (chk, 512)
    cc_cores = []
    for c in range(NCORE):
        both = np.stack([cl[c], cr[c]], axis=0)          # [2,NCHK,128,NBB,512]
        both = both.transpose(3, 0, 2, 1, 4)             # [NBB,2,128,NCHK,512]
        cc_cores.append(np.ascontiguousarray(
            both.reshape(NBB * 2 * 128, NCHK * 512).astype(cnp)))
    del cl, cr

    anc = [np.asarray(inputs["anc_d"]), np.asarray(inputs["anc_p"]), np.asarray(inputs["anc_a"])]
    leaf = [np.asarray(inputs["leaf_d"]), np.asarray(inputs["leaf_p"]), np.asarray(inputs["leaf_a"])]

    def dag_idx(tabs, core):
        out = np.zeros(DAGROWS, np.int64)
        off = 0
        for g in range(3):
            vsl = slice(core * VS[g], (core + 1) * VS[g])
            for l in range(LS[g]):
                out[off:off + VS[g]] = tabs[g][vsl, l] + EOFF[g]
                out[off + VS[g]:off + VPAD[g]] = EOFF[g]
                off += VPAD[g]
        return _wrap_idx(out)

    # per-partition validity mask, one column per v-tile of each group
    maskP = np.zeros((128, 18), np.float32)
    for g in range(3):
        for t in range(NTIL[g]):
            v0 = t * 128
            maskP[:, MOFF[g] + t] = (np.arange(v0, v0 + 128) < VS[g]).astype(np.float32)

    emb16 = np.concatenate([np.asarray(inputs["emb_d"]),
                            np.asarray(inputs["emb_p"]),
                            np.asarray(inputs["emb_a"])], axis=0).astype(np.float16)
    wlA = np.concatenate([np.asarray(inputs[k])[:, :H].T for k in ("Wl_d", "Wl_p", "Wl_a")],
                         axis=1).astype(np.float16)      # [128, 384]
    wlL = np.concatenate([np.asarray(inputs[k])[:, H:].T for k in ("Wl_d", "Wl_p", "Wl_a")],
                         axis=1).astype(np.float16)
    bl3 = np.stack([np.asarray(inputs[k]) for k in ("bl_d", "bl_p", "bl_a")], axis=1).astype(np.float32)
    ap3 = np.concatenate([np.asarray(inputs[k]) for k in ("ap_d", "ap_p", "ap_a")], axis=1).astype(np.float16)
    W_ntn = np.asarray(inputs["W_ntn"]).astype(np.float32)
    wpk = np.concatenate([W_ntn[:, :, p] for p in range(P16)], axis=1).astype(np.float32)  # [128,2048]
    V_ntn = np.asarray(inputs["V_ntn"]).astype(np.float32)
    vlT = np.ascontiguousarray(V_ntn[:, :H].T).astype(np.float32)   # [128,16]
    vrT = np.ascontiguousarray(V_ntn[:, H:].T).astype(np.float32)
    bntr = np.asarray(inputs["b_ntn"]).astype(np.float32).reshape(1, P16).copy()
    wfcc = np.asarray(inputs["w_fc"]).astype(np.float16).reshape(P16, 1).copy()  # [16,1]
    bfc = np.full((1, 1), float(np.asarray(inputs["b_fc"]).reshape(-1)[0]), np.float32)
    onesr = np.ones((1, 512), np.float32)
    ident = np.eye(128, dtype=np.float16)
    # colsel[:, p*16+q] = 1 iff q == p: lhsT that routes a column-sum into row p
    colsel = np.zeros((128, P16 * P16), np.float32)
    for p in range(P16):
        colsel[:, p * P16 + p] = 1.0

    shared = dict(emb16=emb16, wlA=wlA, wlL=wlL, bl3=bl3, ap3=ap3,
                  wpk=wpk, vlT=vlT, vrT=vrT, bntr=bntr, wfcc=wfcc, bfc=bfc,
                  onesr=onesr, ident=ident, colsel=colsel, maskP=maskP)
    in_maps = []
    for c in range(NCORE):
        m = dict(shared)
        m["aidx"] = dag_idx(anc, c)
        m["lidx"] = dag_idx(leaf, c)
        m["cc"] = cc_cores[c]
        in_maps.append(m)

    # ---------------- device program ----------------
    def build(nrep):
        nc = bacc.Bacc("TRN2", target_bir_lowering=False, debug=False,
                       enable_asserts=False, num_devices=NCORE)

        def din(name, arr, dt):
            return nc.dram_tensor(name, list(np.asarray(arr).shape), dt,
                                  kind="ExternalInput").ap()

        d_emb16 = din("emb16", emb16, f16)
        d_wlA = din("wlA", wlA, f16)
        d_wlL = din("wlL", wlL, f16)
        d_bl3 = din("bl3", bl3, f32)
        d_ap3 = din("ap3", ap3, f16)
        d_wpk = din("wpk", wpk, f32)
        d_vlT = din("vlT", vlT, f32)
        d_vrT = din("vrT", vrT, f32)
        d_bntr = din("bntr", bntr, f32)
        d_wfcc = din("wfcc", wfcc, f16)
        d_bfc = din("bfc", bfc, f32)
        d_onesr = din("onesr", onesr, f32)
        d_ident = din("ident", ident, f16)
        d_colsel = din("colsel", colsel, f32)
        d_mask = din("maskP", maskP, f32)
        d_aidx = din("aidx", in_maps[0]["aidx"], i16)
        d_lidx = din("lidx", in_maps[0]["lidx"], i16)
        d_cc = din("cc", in_maps[0]["cc"], f8 if use_f8 else f16)

        d_out = nc.dram_tensor("out", [1, BLOC], f32, kind="ExternalOutput").ap()

        d_sbin = nc.dram_tensor("sbin", [16], f32, kind="Internal").ap()
        d_sbga = nc.dram_tensor("sbga", [NCORE * 16], f32, kind="Internal",
                                addr_space="Shared").ap()
        d_rsin = nc.dram_tensor("rsin", [NBB * 2 * 128, 512], f32, kind="Internal").ap()
        d_rsout = nc.dram_tensor("rsout", [2 * 128, 512], f32, kind="Internal").ap()

        RG = [list(range(NCORE))]
        AT = mybir.ActivationFunctionType
        AL = mybir.AluOpType

        with tile.TileContext(nc) as tc:
            from contextlib import ExitStack
            est = ExitStack()
            with est:
                cpool = est.enter_context(tc.tile_pool(name="consts", bufs=1))
                dagp = est.enter_context(tc.tile_pool(name="dag", bufs=2))
                hpool = est.enter_context(tc.tile_pool(name="hsb", bufs=3))
                smp = est.enter_context(tc.tile_pool(name="smallsb", bufs=4))
                accp = est.enter_context(tc.tile_pool(name="acc", bufs=2))
                embp = est.enter_context(tc.tile_pool(name="embT", bufs=1))
                rhsp = est.enter_context(tc.tile_pool(name="ccrhs", bufs=3))
                segs = est.enter_context(tc.tile_pool(name="segsb", bufs=2))
                hdp = est.enter_context(tc.tile_pool(name="headsb", bufs=4))

                _ldn = [0]
                def load(dram_ap, shape, dt):
                    _ldn[0] += 1
                    t = cpool.tile(shape, dt, tag=f"c{_ldn[0]}", name=f"c{_ldn[0]}")
                    nc.sync.dma_start(out=t[:], in_=dram_ap)
                    return t

                t_wlA = load(d_wlA[:, :], [128, 384], f16)
                t_wlL = load(d_wlL[:, :], [128, 384], f16)
                t_bl3 = load(d_bl3[:, :], [128, 3], f32)
                t_ap3 = load(d_ap3[:, :], [128, 3], f16)
                t_wpk = load(d_wpk[:, :], [128, 2048], f32)
                t_vlT = load(d_vlT[:, :], [128, 16], f32)
                t_vrT = load(d_vrT[:, :], [128, 16], f32)
                t_bntr = load(d_bntr[:, :], [1, 16], f32)
                t_wfcc = load(d_wfcc[:, :], [16, 1], f16)
                t_bfc = load(d_bfc[:, :], [1, 1], f32)
                t_onesr = load(d_onesr[:, :], [1, 512], f32)
                t_ident = load(d_ident[:, :], [128, 128], f16)
                t_colsel = load(d_colsel[:, :], [128, P16 * P16], f32)
                t_mask = load(d_mask[:, :], [128, 18], f32)
                t_aidx = load(d_aidx[:, :], [128, DAGROWS // 16], i16)
                t_lidx = load(d_lidx[:, :], [128, DAGROWS // 16], i16)

                for rep in range(nrep):
                    body(nc, tc, tile, mybir, bass_isa, est, locals())

        nc.compile()
        return nc

    # the per-repetition device program body
    def body(nc, tc, tile, mybir, bass_isa, est, L_):
        from contextlib import ExitStack
        AT = mybir.ActivationFunctionType
        AL = mybir.AluOpType
        RG = [list(range(NCORE))]
        t_wlA, t_wlL = L_["t_wlA"], L_["t_wlL"]
        t_bl3, t_ap3 = L_["t_bl3"], L_["t_ap3"]
        t_wpk, t_vlT, t_vrT = L_["t_wpk"], L_["t_vlT"], L_["t_vrT"]
        t_bntr, t_wfcc, t_bfc = L_["t_bntr"], L_["t_wfcc"], L_["t_bfc"]
        t_onesr, t_ident, t_colsel = L_["t_onesr"], L_["t_ident"], L_["t_colsel"]
        t_mask, t_aidx, t_lidx = L_["t_mask"], L_["t_aidx"], L_["t_lidx"]
        d_emb16, d_cc = L_["d_emb16"], L_["d_cc"]
        d_out, d_sbin, d_sbga = L_["d_out"], L_["d_sbin"], L_["d_sbga"]
        d_rsin, d_rsout = L_["d_rsin"], L_["d_rsout"]
        cpool, dagp, hpool, smp = L_["cpool"], L_["dagp"], L_["hpool"], L_["smp"]
        accp, embp, rhsp, segs, hdp = (L_["accp"], L_["embp"], L_["rhsp"],
                                       L_["segs"], L_["hdp"])
        f32 = mybir.dt.float32
        f16 = mybir.dt.float16
        cdt = mybir.dt.float8e4 if use_f8 else mybir.dt.float16

        if KPH == "Z":
            tz = smp.tile([1, 512], f32, tag="z", name="tz")
            nc.vector.memset(tz[:], 0.0)
            nc.sync.dma_start(out=d_out[:, :], in_=tz[:])
            return

        # ---------- Phase A: attention logits + softmax partials ----------
        # prefetch big C-count DMAs early (independent of everything)
        rhs_tiles = {}
        if KPH in ("F", "E"):
            for pre in range(3):
                bb, side = pre // 2, pre % 2
                rt = rhsp.tile([128, NCHK * 512], cdt, tag="rhs", name="rhs")
                nc.sync.dma_start(
                    out=rt[:], in_=d_cc[(bb * 2 + side) * 128:(bb * 2 + side + 1) * 128, :])
                rhs_tiles[(bb, side)] = rt

        estA = ExitStack()
        ps_h = estA.enter_context(tc.tile_pool(name="psh", bufs=2, space="PSUM"))
        ps_aw = estA.enter_context(tc.tile_pool(name="psaw", bufs=2, space="PSUM"))
        t_sacc = smp.tile([128, 16], f32, tag="sacc", name="t_sacc")
        nc.vector.memset(t_sacc[:], 0.0)
        aT_all = []
        roff = 0
        for g in range(3):
            vp, Lg = VPAD[g], LS[g]
            co = roff // 16
            n = vp * Lg
            aT = dagp.tile([128, 1, n], f16, tag=f"anc{g}", name="aT", bufs=1)
            nc.gpsimd.dma_gather(
                out_ap=aT[:, :, :n], in_ap=d_emb16[:, :],
                idxs_ap=t_aidx[:, co:co + n // 16],
                num_idxs=n, num_idxs_reg=n, elem_size=H, transpose=True,
                single_packet=False, queue_num=0)
            lT = dagp.tile([128, 1, 5120], f16, tag="leaf", name="lT", bufs=2)
            nc.gpsimd.dma_gather(
                out_ap=lT[:, :, :n], in_ap=d_emb16[:, :],
                idxs_ap=t_lidx[:, co:co + n // 16],
                num_idxs=n, num_idxs_reg=n, elem_size=H, transpose=True,
                single_packet=False, queue_num=0)
            aT_all.append(aT)
            if KPH == "G":
                roff += n
                continue
            for c0 in range(0, vp, 512):
                w = min(512, vp - c0)
                nsub = w // 128
                awp = ps_aw.tile([128, 4, 16], f32, tag="awp", name="awp")
                for l in range(Lg):
                    hp = ps_h.tile([128, 512], f32, tag="hp", name="hp")
                    nc.tensor.matmul(hp[:, :w], t_wlA[:, g * 128:(g + 1) * 128],
                                     aT[:, 0, l * vp + c0:l * vp + c0 + w],
                                     start=True, stop=False)
                    nc.tensor.matmul(hp[:, :w], t_wlL[:, g * 128:(g + 1) * 128],
                                     lT[:, 0, l * vp + c0:l * vp + c0 + w],
                                     start=False, stop=True)
                    hs = hpool.tile([128, 512], f16, tag="hs", name="hs")
                    nc.scalar.activation(hs[:, :w], hp[:, :w], AT.Tanh,
                                         bias=t_bl3[:, g:g + 1])
                    for sub in range(nsub):
                        nc.tensor.matmul(awp[:, sub, l:l + 1],
                                         hs[:, sub * 128:(sub + 1) * 128],
                                         t_ap3[:, g:g + 1],
                                         start=True, stop=True)
                for sub in range(nsub):
                    t = c0 // 128 + sub
                    ex = smp.tile([128, 16], f32, tag="ex", name="ex")
                    nc.scalar.activation(ex[:, :Lg], awp[:, sub, :Lg], AT.Exp)
                    den = smp.tile([128, 1], f32, tag="den", name="den")
                    nc.vector.tensor_reduce(den[:], ex[:, :Lg],
                                            axis=mybir.AxisListType.X, op=AL.add)
                    idn = smp.tile([128, 1], f32, tag="idn", name="idn")
                    nc.vector.reciprocal(idn[:], den[:])
                    sm = smp.tile([128, 16], f32, tag="sm", name="sm")
                    nc.vector.tensor_scalar(out=sm[:, :Lg], in0=ex[:, :Lg],
                                            scalar1=idn[:, 0:1], scalar2=None,
                                            op0=AL.mult)
                    smm = smp.tile([128, 16], f32, tag="smm", name="smm")
                    nc.vector.tensor_scalar(out=smm[:, :Lg], in0=sm[:, :Lg],
                                            scalar1=t_mask[:, MOFF[g] + t:MOFF[g] + t + 1],
                                            scalar2=None, op0=AL.mult)
                    nc.vector.tensor_tensor(
                        out=t_sacc[:, GCOL[g]:GCOL[g] + Lg],
                        in0=t_sacc[:, GCOL[g]:GCOL[g] + Lg],
                        in1=smm[:, :Lg], op=AL.add)
            roff += n
        t_sred = smp.tile([128, 16], f32, tag="sred", name="t_sred")
        if KPH not in ("G",):
            nc.gpsimd.partition_all_reduce(t_sred[:], t_sacc[:], channels=128,
                                           reduce_op=bass_isa.ReduceOp.add)
        estA.close()
        if KPH == "G":
            nc.sync.dma_start(out=d_out[0, 0:16], in_=t_sacc[0:1, :])
            return

        # ---------- Phase B: global sbar ----------
        nc.sync.dma_start(out=d_sbin[:], in_=t_sred[0:1, :])
        nc.gpsimd.collective_compute(
            "AllGather", AL.bypass, replica_groups=RG,
            ins=[d_sbin[:]], outs=[d_sbga[:]])
        t_sba = smp.tile([8, 16], f32, tag="sba", name="t_sba")
        nc.sync.dma_start(out=t_sba[:], in_=d_sbga[:])
        t_sbr = smp.tile([8, 16], f32, tag="sbr", name="t_sbr")
        nc.gpsimd.partition_all_reduce(t_sbr[:], t_sba[:], channels=8,
                                       reduce_op=bass_isa.ReduceOp.add)
        t_sbb = smp.tile([128, 16], f32, tag="sbb", name="t_sbb")
        nc.gpsimd.partition_broadcast(t_sbb[:], t_sbr[0:1, :], channels=128)

        if KPH == "A":
            nc.sync.dma_start(out=d_out[0, 0:16], in_=t_sbb[0:1, :])
            return

        # ---------- Phase C: all_emb shard -> [v,h] fp16 lhsT chunks ----------
        estC = ExitStack()
        ps_tr = estC.enter_context(tc.tile_pool(name="pstr", bufs=2, space="PSUM"))
        t_embT = embp.tile([128, SHROWS], f16, tag="embT", name="t_embT")
        for g in range(3):
            vp, Lg = VPAD[g], LS[g]
            aT = aT_all[g]
            acc = accp.tile([128, 1280], f32, tag="acc", name="acc")
            nc.vector.tensor_scalar(out=acc[:, :vp], in0=aT[:, 0, 0:vp],
                                    scalar1=t_sbb[:, GCOL[g]:GCOL[g] + 1],
                                    scalar2=None, op0=AL.mult)
            for l in range(1, Lg):
                tmp = accp.tile([128, 1280], f32, tag="tmp", name="tmp")
                nc.vector.tensor_scalar(out=tmp[:, :vp], in0=aT[:, 0, l * vp:(l + 1) * vp],
                                        scalar1=t_sbb[:, GCOL[g] + l:GCOL[g] + l + 1],
                                        scalar2=None, op0=AL.mult)
                nc.vector.tensor_tensor(out=acc[:, :vp], in0=acc[:, :vp],
                                        in1=tmp[:, :vp], op=AL.add)
            acch = hpool.tile([128, 1280], f16, tag="acch", name="acch")
            nc.scalar.activation(acch[:, :vp], acc[:, :vp], AT.Copy)
            for t in range(NTIL[g]):
                pst = ps_tr.tile([128, 128], f16, tag="pst", name="pst")
                nc.tensor.transpose(pst[:], acch[:, t * 128:(t + 1) * 128],
                                    t_ident[:])
                r0 = GOFF_SH[g] + t * 128
                nc.vector.tensor_copy(t_embT[:, r0:r0 + 128], pst[:])
        estC.close()
        if KPH == "C":
            nc.sync.dma_start(out=d_out[0, 0:128], in_=t_embT[0:1, 0:128])
            return

        # ---------- Phase E: count matmuls + ReduceScatter ----------
        estE = ExitStack()
        ps_seg = estE.enter_context(tc.tile_pool(name="psseg", bufs=3, space="PSUM"))
        for bb in range(NBB):
            for side in range(2):
                key = (bb, side)
                if key in rhs_tiles:
                    rt = rhs_tiles.pop(key)
                else:
                    rt = rhsp.tile([128, NCHK * 512], cdt, tag="rhs", name="rhs")
                    nc.sync.dma_start(
                        out=rt[:],
                        in_=d_cc[(bb * 2 + side) * 128:(bb * 2 + side + 1) * 128, :])
                ps = ps_seg.tile([128, 512], f32, tag="pseg", name="pseg")
                for chk in range(NCHK):
                    nc.tensor.matmul(ps[:], t_embT[:, chk * 128:(chk + 1) * 128],
                                     rt[:, chk * 512:(chk + 1) * 512],
                                     start=(chk == 0), stop=(chk == NCHK - 1))
                sb = segs.tile([128, 512], f32, tag="segsb", name="sb", bufs=4)
                if (bb + side) % 2 == 0:
                    nc.scalar.activation(sb[:], ps[:], AT.Copy)
                else:
                    nc.vector.tensor_copy(sb[:], ps[:])
                nc.sync.dma_start(
                    out=d_rsin[(bb * 2 + side) * 128:(bb * 2 + side + 1) * 128, :],
                    in_=sb[:])
        estE.close()
        nc.gpsimd.collective_compute(
            "ReduceScatter", AL.add, replica_groups=RG,
            ins=[d_rsin[:, :]], outs=[d_rsout[:, :]])

        leT = segs.tile([128, 512], f32, tag="leT", name="leT")
        nc.sync.dma_start(out=leT[:], in_=d_rsout[0:128, :])
        reT = segs.tile([128, 512], f32, tag="reT", name="reT")
        nc.sync.dma_start(out=reT[:], in_=d_rsout[128:256, :])
        if KPH == "E":
            nc.sync.dma_start(out=d_out[:, :], in_=leT[0:1, :])
            return

        # ---------- Phase F: NTN head ----------
        estF = ExitStack()
        ps_hd = estF.enter_context(tc.tile_pool(name="pshd", bufs=2, space="PSUM"))
        ps_16 = estF.enter_context(tc.tile_pool(name="ps16", bufs=1, space="PSUM"))
        ps_out = estF.enter_context(tc.tile_pool(name="psout", bufs=1, space="PSUM"))

        mls = []
        for p in range(P16):
            tp = ps_hd.tile([128, 512], f32, tag="tp", name="tp")
            nc.tensor.matmul(tp[:], t_wpk[:, p * 128:(p + 1) * 128], leT[:],
                             start=True, stop=True)
            ml = hdp.tile([128, 512], f32, tag=f"ml{p}", name="ml", bufs=1)
            nc.vector.tensor_tensor(out=ml[:], in0=tp[:], in1=reT[:], op=AL.mult)
            mls.append(ml)
        # pair_sim pre-activation rows: V@[le;re] + b + bilinear, all in PSUM
        ps16 = ps_16.tile([16, 512], f32, tag="ps16", name="ps16")
        nc.tensor.matmul(ps16[:], t_vlT[:, :], leT[:], start=True, stop=False)
        nc.tensor.matmul(ps16[:], t_vrT[:, :], reT[:], start=False, stop=False)
        nc.tensor.matmul(ps16[:], t_bntr[:, :], t_onesr[:, :], start=False, stop=False)
        for p in range(P16):
            nc.tensor.matmul(ps16[:], t_colsel[:, p * P16:(p + 1) * P16], mls[p][:],
                             start=False, stop=(p == P16 - 1))
        th = hdp.tile([16, 512], f16, tag="th", name="th")
        nc.scalar.activation(th[:], ps16[:], AT.Tanh)
        pso = ps_out.tile([1, 512], f32, tag="pso", name="pso")
        nc.tensor.matmul(pso[:], t_wfcc[:, :], th[:], start=True, stop=True)
        sg = hdp.tile([1, 512], f32, tag="sg", name="sg")
        nc.scalar.activation(sg[:], pso[:], AT.Sigmoid, bias=t_bfc[:, 0:1])
        estF.close()
        nc.sync.dma_start(out=d_out[:, :], in_=sg[:])

    nc1 = build(1)
    _trace_kw = {}
    if os.environ.get("KTRACE"):
        _trace_kw = dict(trace=True, tmpdir=os.environ.get("KTRACEDIR") or None)
    res = run_bass_kernel_spmd(nc1, in_maps, list(range(NCORE)), **_trace_kw)
    global LAST_RESULT, LAST_EXEC_NS
    LAST_RESULT = res

    if os.environ.get("KTIME", "1") != "0":
        import time as _time
        try:
            import jax
            from jax.sharding import Mesh, PartitionSpec, NamedSharding
            from jax.experimental.shard_map import shard_map
            import concourse.mybir as mybir2
            from concourse import bass2jax as b2j
            b2j.install_neuronx_cc_hook()
            _conc_cache = {}

            def time_program(nc):
                in_names, out_names, out_avals, zero_outs = [], [], [], []
                pname = nc.partition_id_tensor.name if nc.partition_id_tensor else None
                for alloc in nc.m.functions[0].allocations:
                    if not isinstance(alloc, mybir2.MemoryLocationSet):
                        continue
                    name = alloc.memorylocations[0].name
                    if alloc.kind == "ExternalInput":
                        if name != pname:
                            in_names.append(name)
                    elif alloc.kind == "ExternalOutput":
                        shape = tuple(alloc.tensor_shape)
                        dtype = mybir2.dt.np(alloc.dtype)
                        out_names.append(name)
                        out_avals.append(jax.core.ShapedArray(shape, dtype))
                        zero_outs.append(np.zeros(shape, dtype))
                n_params = len(in_names)
                all_in = list(in_names) + list(out_names)
                if pname is not None:
                    all_in.append(pname)

                def _body(*args):
                    ops = list(args)
                    if pname is not None:
                        ops.append(b2j.partition_id_tensor())
                    return tuple(b2j._bass_exec_p.bind(
                        *ops, out_avals=tuple(out_avals), in_names=tuple(all_in),
                        out_names=tuple(out_names),
                        lowering_input_output_aliases=(),
                        sim_require_finite=True, sim_require_nnan=True, nc=nc))

                devices = jax.devices()[:NCORE]
                mesh = Mesh(np.asarray(devices), ("core",))
                nio = n_params + len(out_names)
                fn = jax.jit(shard_map(_body, mesh=mesh,
                                       in_specs=(PartitionSpec("core"),) * nio,
                                       out_specs=(PartitionSpec("core"),) * len(out_names),
                                       check_rep=False),
                             donate_argnums=tuple(range(n_params, nio)),
                             keep_unused=True)
                sh = NamedSharding(mesh, PartitionSpec("core"))
                ckey = tuple(in_names)
                if ckey not in _conc_cache:
                    _conc_cache[ckey] = [jax.device_put(np.concatenate(
                        [np.asarray(in_maps[c][n]) for c in range(NCORE)], axis=0), sh)
                        for n in in_names]
                conc = _conc_cache[ckey]
                NIT = int(os.environ.get("KITER", "6"))
                BURST = int(os.environ.get("KBURST", "32"))
                best = None
                _times = []
                for it in range(NIT):
                    zss = [[jax.device_put(
                              np.zeros((NCORE * z.shape[0], *z.shape[1:]), z.dtype), sh)
                            for z in zero_outs] for _ in range(BURST)]
                    jax.block_until_ready(fn(*conc, *zss[0]))
                    t0 = _time.perf_counter()
                    outs = [fn(*conc, *zs) for zs in zss[1:]]
                    jax.block_until_ready(outs)
                    dt = (_time.perf_counter() - t0) / max(1, BURST - 1)
                    _times.append(dt)
                    if it > 0:
                        best = dt if best is None else min(best, dt)
                if os.environ.get("KVERBOSE"):
                    print("per-call times (ms):", [round(t * 1e3, 3) for t in _times])
                return best

            t1 = time_program(nc1)
            R = int(os.environ.get("KREP", "8"))
            if R > 1:
                ncR = build(R)
                tR = time_program(ncR)
                exec_s = max((tR - t1) / (R - 1), 1e-9)
                if os.environ.get("KVERBOSE"):
                    print(f"t1={t1*1e3:.3f}ms tR={tR*1e3:.3f}ms "
                          f"-> per-exec {(tR-t1)/(R-1)*1e3:.3f}ms")
            else:
                exec_s = t1
            LAST_EXEC_NS = int(exec_s * 1e9)
        except Exception as e:
            import traceback
            traceback.print_exc()
            print("KTIME path failed:", repr(e))
    outs = [np.asarray(res.results[c]["out"]).reshape(BLOC) for c in range(NCORE)]
    return np.concatenate(outs).astype(np.float32)


if __name__ == "__main__":
    pass


# revision 40
# speedup vs baseline: 1.2578x; 1.2578x over previous
"""GRAM model Trainium2 kernel: 8-core SPMD via bass/tile.

Strategy (data-parallel over graphs for the NTN head, vocab-parallel for
the DAG-embedding stage, per the sharding hint):

 - DAG embedding stage sharded over vocab (exact /8 shards): fp16
   transposed gathers (one per group x {anc,leaf}) feed PE matmuls for
   h=tanh(cat@Wl.T+bl); attention logits in [v,l] layout via lhsT=h
   matmuls; softmax per v-tile; global softmax weight sums (13 floats)
   via AllGather + on-chip reduce.
 - all_emb shard is built by re-using the SBUF-resident anc tiles
   (weighted sum over levels with the global sbar weights), then PE
   transposes into [v,h] fp16 lhsT chunks.  No second gather pass.
 - segment-sum + node gather are fused into a count matmul:
   le.T[h,b] = sum_v emb_shard[v,h] * C[v,b], where C is the (vocab-row,
   graph) multiplicity matrix built host-side from the integer index
   tensors.  Each core contracts its own 2304 vocab rows against all
   4096 graphs (streamed from HBM in fp16), and one ReduceScatter(add)
   both sums the partials over cores and leaves each core exactly its
   512-graph block of le/re -- no all_embedding AllGather, no per-node
   gather descriptors.
 - NTN head computed per core on its 512 graph pairs.

Timing: the printed HW exec time is measured differentially -- the same
program is compiled once with the body repeated KREP times on-device and
once plain; (T_rep - T_plain)/(KREP-1) under pipelined dispatch isolates
the on-device execution time from the ~2-70ms host->device dispatch
latency of this environment (an empty kernel measures the same as the
full one in a naive per-call measurement).
"""
import os
import numpy as np

KPH = os.environ.get("KPH", "F")
LAST_RESULT = None
LAST_EXEC_NS = None

H = 128
P16 = 16
B = 4096
T = 262144
V_D, V_P, V_A = 10000, 4000, 4000
LS = [4, 4, 5]
NCORE = 8
BLOC = B // NCORE          # 512 graph pairs per core
VS = [1250, 500, 500]
VPAD = [1280, 512, 512]
NTIL = [10, 4, 4]
MOFF = [0, 10, 14]         # tile-column offsets into the mask array
GCOL = [0, 4, 8]           # sbar column offsets per group
GOFF_SH = [0, 1280, 1792]  # row offset of group inside a rank's shard
SHROWS = 2304              # rows per rank shard (incl pads)
EOFF = [0, 13000, 18200]   # group offsets in emb_cat (23400 rows)
NCHK = SHROWS // 128       # 18 lhsT chunks per core
NBB = B // 512             # 8 graph blocks of 512
DAGROWS = sum(VPAD[g] * LS[g] for g in range(3))   # 9728


def _build_perm():
    perm = np.empty(18000, np.int64)
    v = np.arange(V_D)
    perm[:V_D] = (v // VS[0]) * SHROWS + GOFF_SH[0] + (v % VS[0])
    v = np.arange(V_P)
    perm[V_D:V_D + V_P] = (v // VS[1]) * SHROWS + GOFF_SH[1] + (v % VS[1])
    v = np.arange(V_A)
    perm[V_D + V_P:] = (v // VS[2]) * SHROWS + GOFF_SH[2] + (v % VS[2])
    return perm


def _wrap_idx(a):
    """dma_gather index layout: element i at [i%16, i//16]; replicate to 128 parts."""
    m = a.reshape(-1, 16).T.astype(np.int16)
    return np.ascontiguousarray(np.tile(m, (8, 1)))


def kernel(**inputs):
    import concourse.bacc as bacc
    import concourse.tile as tile
    import concourse.mybir as mybir
    from concourse import bass_isa
    from concourse.bass_utils import run_bass_kernel_spmd

    f32 = mybir.dt.float32
    f16 = mybir.dt.float16
    i16 = mybir.dt.int16
    f8 = mybir.dt.float8e4

    # ---------------- host-side shard prep ----------------
    lx = np.asarray(inputs["left_x"])[:, 0].astype(np.int64)
    rx = np.asarray(inputs["right_x"])[:, 0].astype(np.int64)
    lb = np.asarray(inputs["left_x_batch"]).astype(np.int64)
    rb = np.asarray(inputs["right_x_batch"]).astype(np.int64)

    perm = _build_perm()

    def count_mats(pos, seg):
        """[core, chk, row, bb, col] multiplicity counts."""
        cnt = np.bincount(pos * B + seg, minlength=NCORE * SHROWS * B)
        mx = int(cnt.max())
        assert mx < 2048, "counts exceed fp16 exact-integer range"
        return cnt.astype(np.float16).reshape(NCORE, NCHK, 128, NBB, 512), mx

    cl, mxl = count_mats(perm[lx], lb)
    cr, mxr = count_mats(perm[rx], rb)
    # fp8e4m3 represents integers exactly up to 16; halves the HBM stream
    use_f8 = max(mxl, mxr) <= 16 and os.environ.get("KC8", "1") != "0"
    import ml_dtypes
    cnp = ml_dtypes.float8_e4m3 if use_f8 else np.float16
    # per-core layout: rows (bb, side, 128), cols (chk, 512)
    cc_cores = []
    for c in range(NCORE):
        both = np.stack([cl[c], cr[c]], axis=0)          # [2,NCHK,128,NBB,512]
        both = both.transpose(3, 0, 2, 1, 4)             # [NBB,2,128,NCHK,512]
        cc_cores.append(np.ascontiguousarray(
            both.reshape(NBB * 2 * 128, NCHK * 512).astype(cnp)))
    del cl, cr

    anc = [np.asarray(inputs["anc_d"]), np.asarray(inputs["anc_p"]), np.asarray(inputs["anc_a"])]
    leaf = [np.asarray(inputs["leaf_d"]), np.asarray(inputs["leaf_p"]), np.asarray(inputs["leaf_a"])]

    def dag_idx(tabs, core):
        out = np.zeros(DAGROWS, np.int64)
        off = 0
        for g in range(3):
            vsl = slice(core * VS[g], (core + 1) * VS[g])
            for l in range(LS[g]):
                out[off:off + VS[g]] = tabs[g][vsl, l] + EOFF[g]
                out[off + VS[g]:off + VPAD[g]] = EOFF[g]
                off += VPAD[g]
        return _wrap_idx(out)

    # per-partition validity mask, one column per v-tile of each group
    maskP = np.zeros((128, 18), np.float32)
    for g in range(3):
        for t in range(NTIL[g]):
            v0 = t * 128
            maskP[:, MOFF[g] + t] = (np.arange(v0, v0 + 128) < VS[g]).astype(np.float32)

    emb16 = np.concatenate([np.asarray(inputs["emb_d"]),
                            np.asarray(inputs["emb_p"]),
                            np.asarray(inputs["emb_a"])], axis=0).astype(np.float16)
    wlA = np.concatenate([np.asarray(inputs[k])[:, :H].T for k in ("Wl_d", "Wl_p", "Wl_a")],
                         axis=1).astype(np.float16)      # [128, 384]
    wlL = np.concatenate([np.asarray(inputs[k])[:, H:].T for k in ("Wl_d", "Wl_p", "Wl_a")],
                         axis=1).astype(np.float16)
    bl3 = np.stack([np.asarray(inputs[k]) for k in ("bl_d", "bl_p", "bl_a")], axis=1).astype(np.float32)
    ap3 = np.concatenate([np.asarray(inputs[k]) for k in ("ap_d", "ap_p", "ap_a")], axis=1).astype(np.float16)
    W_ntn = np.asarray(inputs["W_ntn"]).astype(np.float32)
    wpk = np.concatenate([W_ntn[:, :, p] for p in range(P16)], axis=1).astype(np.float32)  # [128,2048]
    V_ntn = np.asarray(inputs["V_ntn"]).astype(np.float32)
    vlT = np.ascontiguousarray(V_ntn[:, :H].T).astype(np.float32)   # [128,16]
    vrT = np.ascontiguousarray(V_ntn[:, H:].T).astype(np.float32)
    bntr = np.asarray(inputs["b_ntn"]).astype(np.float32).reshape(1, P16).copy()
    wfcc = np.asarray(inputs["w_fc"]).astype(np.float16).reshape(P16, 1).copy()  # [16,1]
    bfc = np.full((1, 1), float(np.asarray(inputs["b_fc"]).reshape(-1)[0]), np.float32)
    onesr = np.ones((1, 512), np.float32)
    ident = np.eye(128, dtype=np.float16)
    # colsel[:, p*16+q] = 1 iff q == p: lhsT that routes a column-sum into row p
    colsel = np.zeros((128, P16 * P16), np.float32)
    for p in range(P16):
        colsel[:, p * P16 + p] = 1.0

    shared = dict(emb16=emb16, wlA=wlA, wlL=wlL, bl3=bl3, ap3=ap3,
                  wpk=wpk, vlT=vlT, vrT=vrT, bntr=bntr, wfcc=wfcc, bfc=bfc,
                  onesr=onesr, ident=ident, colsel=colsel, maskP=maskP)
    in_maps = []
    for c in range(NCORE):
        m = dict(shared)
        m["aidx"] = dag_idx(anc, c)
        m["lidx"] = dag_idx(leaf, c)
        m["cc"] = cc_cores[c]
        in_maps.append(m)

    # ---------------- device program ----------------
    def build(nrep):
        nc = bacc.Bacc("TRN2", target_bir_lowering=False, debug=False,
                       enable_asserts=False, num_devices=NCORE)

        def din(name, arr, dt):
            return nc.dram_tensor(name, list(np.asarray(arr).shape), dt,
                                  kind="ExternalInput").ap()

        d_emb16 = din("emb16", emb16, f16)
        d_wlA = din("wlA", wlA, f16)
        d_wlL = din("wlL", wlL, f16)
        d_bl3 = din("bl3", bl3, f32)
        d_ap3 = din("ap3", ap3, f16)
        d_wpk = din("wpk", wpk, f32)
        d_vlT = din("vlT", vlT, f32)
        d_vrT = din("vrT", vrT, f32)
        d_bntr = din("bntr", bntr, f32)
        d_wfcc = din("wfcc", wfcc, f16)
        d_bfc = din("bfc", bfc, f32)
        d_onesr = din("onesr", onesr, f32)
        d_ident = din("ident", ident, f16)
        d_colsel = din("colsel", colsel, f32)
        d_mask = din("maskP", maskP, f32)
        d_aidx = din("aidx", in_maps[0]["aidx"], i16)
        d_lidx = din("lidx", in_maps[0]["lidx"], i16)
        d_cc = din("cc", in_maps[0]["cc"], f8 if use_f8 else f16)

        d_out = nc.dram_tensor("out", [1, BLOC], f32, kind="ExternalOutput").ap()

        d_sbin = nc.dram_tensor("sbin", [16], f32, kind="Internal").ap()
        d_sbga = nc.dram_tensor("sbga", [NCORE * 16], f32, kind="Internal",
                                addr_space="Shared").ap()
        d_rsin = nc.dram_tensor("rsin", [NBB * 2 * 128, 512], f32, kind="Internal").ap()
        d_rsout = nc.dram_tensor("rsout", [2 * 128, 512], f32, kind="Internal").ap()

        RG = [list(range(NCORE))]
        AT = mybir.ActivationFunctionType
        AL = mybir.AluOpType

        with tile.TileContext(nc) as tc:
            from contextlib import ExitStack
            est = ExitStack()
            with est:
                cpool = est.enter_context(tc.tile_pool(name="consts", bufs=1))
                dagp = est.enter_context(tc.tile_pool(name="dag", bufs=2))
                hpool = est.enter_context(tc.tile_pool(name="hsb", bufs=3))
                smp = est.enter_context(tc.tile_pool(name="smallsb", bufs=4))
                accp = est.enter_context(tc.tile_pool(name="acc", bufs=2))
                embp = est.enter_context(tc.tile_pool(name="embT", bufs=1))
                rhsp = est.enter_context(tc.tile_pool(name="ccrhs", bufs=3))
                segs = est.enter_context(tc.tile_pool(name="segsb", bufs=2))
                hdp = est.enter_context(tc.tile_pool(name="headsb", bufs=4))

                _ldn = [0]
                def load(dram_ap, shape, dt):
                    _ldn[0] += 1
                    t = cpool.tile(shape, dt, tag=f"c{_ldn[0]}", name=f"c{_ldn[0]}")
                    nc.sync.dma_start(out=t[:], in_=dram_ap)
                    return t

                t_wlA = load(d_wlA[:, :], [128, 384], f16)
                t_wlL = load(d_wlL[:, :], [128, 384], f16)
                t_bl3 = load(d_bl3[:, :], [128, 3], f32)
                t_ap3 = load(d_ap3[:, :], [128, 3], f16)
                t_wpk = load(d_wpk[:, :], [128, 2048], f32)
                t_vlT = load(d_vlT[:, :], [128, 16], f32)
                t_vrT = load(d_vrT[:, :], [128, 16], f32)
                t_bntr = load(d_bntr[:, :], [1, 16], f32)
                t_wfcc = load(d_wfcc[:, :], [16, 1], f16)
                t_bfc = load(d_bfc[:, :], [1, 1], f32)
                t_onesr = load(d_onesr[:, :], [1, 512], f32)
                t_ident = load(d_ident[:, :], [128, 128], f16)
                t_colsel = load(d_colsel[:, :], [128, P16 * P16], f32)
                t_mask = load(d_mask[:, :], [128, 18], f32)
                t_aidx = load(d_aidx[:, :], [128, DAGROWS // 16], i16)
                t_lidx = load(d_lidx[:, :], [128, DAGROWS // 16], i16)

                for rep in range(nrep):
                    body(nc, tc, tile, mybir, bass_isa, est, locals())

        nc.compile()
        return nc

    # the per-repetition device program body
    def body(nc, tc, tile, mybir, bass_isa, est, L_):
        from contextlib import ExitStack
        AT = mybir.ActivationFunctionType
        AL = mybir.AluOpType
        RG = [list(range(NCORE))]
        t_wlA, t_wlL = L_["t_wlA"], L_["t_wlL"]
        t_bl3, t_ap3 = L_["t_bl3"], L_["t_ap3"]
        t_wpk, t_vlT, t_vrT = L_["t_wpk"], L_["t_vlT"], L_["t_vrT"]
        t_bntr, t_wfcc, t_bfc = L_["t_bntr"], L_["t_wfcc"], L_["t_bfc"]
        t_onesr, t_ident, t_colsel = L_["t_onesr"], L_["t_ident"], L_["t_colsel"]
        t_mask, t_aidx, t_lidx = L_["t_mask"], L_["t_aidx"], L_["t_lidx"]
        d_emb16, d_cc = L_["d_emb16"], L_["d_cc"]
        d_out, d_sbin, d_sbga = L_["d_out"], L_["d_sbin"], L_["d_sbga"]
        d_rsin, d_rsout = L_["d_rsin"], L_["d_rsout"]
        cpool, dagp, hpool, smp = L_["cpool"], L_["dagp"], L_["hpool"], L_["smp"]
        accp, embp, rhsp, segs, hdp = (L_["accp"], L_["embp"], L_["rhsp"],
                                       L_["segs"], L_["hdp"])
        f32 = mybir.dt.float32
        f16 = mybir.dt.float16
        cdt = mybir.dt.float8e4 if use_f8 else mybir.dt.float16

        if KPH == "Z":
            tz = smp.tile([1, 512], f32, tag="z", name="tz")
            nc.vector.memset(tz[:], 0.0)
            nc.sync.dma_start(out=d_out[:, :], in_=tz[:])
            return

        # ---------- Phase A: attention logits + softmax partials ----------
        # prefetch big C-count DMAs early (independent of everything)
        rhs_tiles = {}
        if KPH in ("F", "E"):
            for pre in range(3):
                bb, side = pre // 2, pre % 2
                rt = rhsp.tile([128, NCHK * 512], cdt, tag="rhs", name="rhs")
                nc.sync.dma_start(
                    out=rt[:], in_=d_cc[(bb * 2 + side) * 128:(bb * 2 + side + 1) * 128, :])
                rhs_tiles[(bb, side)] = rt

        estA = ExitStack()
        ps_h = estA.enter_context(tc.tile_pool(name="psh", bufs=2, space="PSUM"))
        ps_aw = estA.enter_context(tc.tile_pool(name="psaw", bufs=2, space="PSUM"))
        t_sacc = smp.tile([128, 16], f32, tag="sacc", name="t_sacc")
        nc.vector.memset(t_sacc[:], 0.0)
        aT_all = []
        roff = 0
        for g in range(3):
            vp, Lg = VPAD[g], LS[g]
            co = roff // 16
            n = vp * Lg
            aT = dagp.tile([128, 1, n], f16, tag=f"anc{g}", name="aT", bufs=1)
            nc.gpsimd.dma_gather(
                out_ap=aT[:, :, :n], in_ap=d_emb16[:, :],
                idxs_ap=t_aidx[:, co:co + n // 16],
                num_idxs=n, num_idxs_reg=n, elem_size=H, transpose=True,
                single_packet=False, queue_num=0)
            lT = dagp.tile([128, 1, 5120], f16, tag="leaf", name="lT", bufs=2)
            nc.gpsimd.dma_gather(
                out_ap=lT[:, :, :n], in_ap=d_emb16[:, :],
                idxs_ap=t_lidx[:, co:co + n // 16],
                num_idxs=n, num_idxs_reg=n, elem_size=H, transpose=True,
                single_packet=False, queue_num=0)
            aT_all.append(aT)
            if KPH == "G":
                roff += n
                continue
            for c0 in range(0, vp, 512):
                w = min(512, vp - c0)
                nsub = w // 128
                awp = ps_aw.tile([128, 4, 16], f32, tag="awp", name="awp")
                for l in range(Lg):
                    hp = ps_h.tile([128, 512], f32, tag="hp", name="hp")
                    nc.tensor.matmul(hp[:, :w], t_wlA[:, g * 128:(g + 1) * 128],
                                     aT[:, 0, l * vp + c0:l * vp + c0 + w],
                                     start=True, stop=False)
                    nc.tensor.matmul(hp[:, :w], t_wlL[:, g * 128:(g + 1) * 128],
                                     lT[:, 0, l * vp + c0:l * vp + c0 + w],
                                     start=False, stop=True)
                    hs = hpool.tile([128, 512], f16, tag="hs", name="hs")
                    nc.scalar.activation(hs[:, :w], hp[:, :w], AT.Tanh,
                                         bias=t_bl3[:, g:g + 1])
                    for sub in range(nsub):
                        nc.tensor.matmul(awp[:, sub, l:l + 1],
                                         hs[:, sub * 128:(sub + 1) * 128],
                                         t_ap3[:, g:g + 1],
                                         start=True, stop=True)
                for sub in range(nsub):
                    t = c0 // 128 + sub
                    ex = smp.tile([128, 16], f32, tag="ex", name="ex")
                    nc.scalar.activation(ex[:, :Lg], awp[:, sub, :Lg], AT.Exp)
                    den = smp.tile([128, 1], f32, tag="den", name="den")
                    nc.vector.tensor_reduce(den[:], ex[:, :Lg],
                                            axis=mybir.AxisListType.X, op=AL.add)
                    idn = smp.tile([128, 1], f32, tag="idn", name="idn")
                    nc.vector.reciprocal(idn[:], den[:])
                    sm = smp.tile([128, 16], f32, tag="sm", name="sm")
                    nc.vector.tensor_scalar(out=sm[:, :Lg], in0=ex[:, :Lg],
                                            scalar1=idn[:, 0:1], scalar2=None,
                                            op0=AL.mult)
                    smm = smp.tile([128, 16], f32, tag="smm", name="smm")
                    nc.vector.tensor_scalar(out=smm[:, :Lg], in0=sm[:, :Lg],
                                            scalar1=t_mask[:, MOFF[g] + t:MOFF[g] + t + 1],
                                            scalar2=None, op0=AL.mult)
                    nc.vector.tensor_tensor(
                        out=t_sacc[:, GCOL[g]:GCOL[g] + Lg],
                        in0=t_sacc[:, GCOL[g]:GCOL[g] + Lg],
                        in1=smm[:, :Lg], op=AL.add)
            roff += n
        t_sred = smp.tile([128, 16], f32, tag="sred", name="t_sred")
        if KPH not in ("G",):
            nc.gpsimd.partition_all_reduce(t_sred[:], t_sacc[:], channels=128,
                                           reduce_op=bass_isa.ReduceOp.add)
        estA.close()
        if KPH == "G":
            nc.sync.dma_start(out=d_out[0, 0:16], in_=t_sacc[0:1, :])
            return

        # ---------- Phase B: global sbar ----------
        nc.sync.dma_start(out=d_sbin[:], in_=t_sred[0:1, :])
        nc.gpsimd.collective_compute(
            "AllGather", AL.bypass, replica_groups=RG,
            ins=[d_sbin[:]], outs=[d_sbga[:]])
        t_sba = smp.tile([8, 16], f32, tag="sba", name="t_sba")
        nc.sync.dma_start(out=t_sba[:], in_=d_sbga[:])
        t_sbr = smp.tile([8, 16], f32, tag="sbr", name="t_sbr")
        nc.gpsimd.partition_all_reduce(t_sbr[:], t_sba[:], channels=8,
                                       reduce_op=bass_isa.ReduceOp.add)
        t_sbb = smp.tile([128, 16], f32, tag="sbb", name="t_sbb")
        nc.gpsimd.partition_broadcast(t_sbb[:], t_sbr[0:1, :], channels=128)

        if KPH == "A":
            nc.sync.dma_start(out=d_out[0, 0:16], in_=t_sbb[0:1, :])
            return

        # ---------- Phase C: all_emb shard -> [v,h] fp16 lhsT chunks ----------
        estC = ExitStack()
        ps_tr = estC.enter_context(tc.tile_pool(name="pstr", bufs=2, space="PSUM"))
        t_embT = embp.tile([128, SHROWS], f16, tag="embT", name="t_embT")
        for g in range(3):
            vp, Lg = VPAD[g], LS[g]
            aT = aT_all[g]
            acc = accp.tile([128, 1280], f32, tag="acc", name="acc")
            nc.vector.tensor_scalar(out=acc[:, :vp], in0=aT[:, 0, 0:vp],
                                    scalar1=t_sbb[:, GCOL[g]:GCOL[g] + 1],
                                    scalar2=None, op0=AL.mult)
            for l in range(1, Lg):
                tmp = accp.tile([128, 1280], f32, tag="tmp", name="tmp")
                nc.vector.tensor_scalar(out=tmp[:, :vp], in0=aT[:, 0, l * vp:(l + 1) * vp],
                                        scalar1=t_sbb[:, GCOL[g] + l:GCOL[g] + l + 1],
                                        scalar2=None, op0=AL.mult)
                nc.vector.tensor_tensor(out=acc[:, :vp], in0=acc[:, :vp],
                                        in1=tmp[:, :vp], op=AL.add)
            acch = hpool.tile([128, 1280], f16, tag="acch", name="acch")
            nc.scalar.activation(acch[:, :vp], acc[:, :vp], AT.Copy)
            for t in range(NTIL[g]):
                pst = ps_tr.tile([128, 128], f16, tag="pst", name="pst")
                nc.tensor.transpose(pst[:], acch[:, t * 128:(t + 1) * 128],
                                    t_ident[:])
                r0 = GOFF_SH[g] + t * 128
                nc.vector.tensor_copy(t_embT[:, r0:r0 + 128], pst[:])
        estC.close()
        if KPH == "C":
            nc.gpsimd.dma_start(out=d_out[0, 0:128], in_=t_embT[0:1, 0:128])
            return

        # ---------- Phase E: count matmuls + ReduceScatter ----------
        estE = ExitStack()
        ps_seg = estE.enter_context(tc.tile_pool(name="psseg", bufs=3, space="PSUM"))
        for bb in range(NBB):
            for side in range(2):
                key = (bb, side)
                if key in rhs_tiles:
                    rt = rhs_tiles.pop(key)
                else:
                    rt = rhsp.tile([128, NCHK * 512], cdt, tag="rhs", name="rhs")
                    nc.sync.dma_start(
                        out=rt[:],
                        in_=d_cc[(bb * 2 + side) * 128:(bb * 2 + side + 1) * 128, :])
                ps = ps_seg.tile([128, 512], f32, tag="pseg", name="pseg")
                for chk in range(NCHK):
                    nc.tensor.matmul(ps[:], t_embT[:, chk * 128:(chk + 1) * 128],
                                     rt[:, chk * 512:(chk + 1) * 512],
                                     start=(chk == 0), stop=(chk == NCHK - 1))
                sb = segs.tile([128, 512], f32, tag="segsb", name="sb", bufs=4)
                if (bb + side) % 2 == 0:
                    nc.scalar.activation(sb[:], ps[:], AT.Copy)
                else:
                    nc.vector.tensor_copy(sb[:], ps[:])
                nc.sync.dma_start(
                    out=d_rsin[(bb * 2 + side) * 128:(bb * 2 + side + 1) * 128, :],
                    in_=sb[:])
        estE.close()
        nc.gpsimd.collective_compute(
            "ReduceScatter", AL.add, replica_groups=RG,
            ins=[d_rsin[:, :]], outs=[d_rsout[:, :]])

        leT = segs.tile([128, 512], f32, tag="leT", name="leT")
        nc.sync.dma_start(out=leT[:], in_=d_rsout[0:128, :])
        reT = segs.tile([128, 512], f32, tag="reT", name="reT")
        nc.sync.dma_start(out=reT[:], in_=d_rsout[128:256, :])
        if KPH == "E":
            nc.sync.dma_start(out=d_out[:, :], in_=leT[0:1, :])
            return

        # ---------- Phase F: NTN head ----------
        estF = ExitStack()
        ps_hd = estF.enter_context(tc.tile_pool(name="pshd", bufs=2, space="PSUM"))
        ps_16 = estF.enter_context(tc.tile_pool(name="ps16", bufs=1, space="PSUM"))
        ps_out = estF.enter_context(tc.tile_pool(name="psout", bufs=1, space="PSUM"))

        mls = []
        for p in range(P16):
            tp = ps_hd.tile([128, 512], f32, tag="tp", name="tp")
            nc.tensor.matmul(tp[:], t_wpk[:, p * 128:(p + 1) * 128], leT[:],
                             start=True, stop=True)
            ml = hdp.tile([128, 512], f32, tag=f"ml{p}", name="ml", bufs=1)
            nc.vector.tensor_tensor(out=ml[:], in0=tp[:], in1=reT[:], op=AL.mult)
            mls.append(ml)
        # pair_sim pre-activation rows: V@[le;re] + b + bilinear, all in PSUM
        ps16 = ps_16.tile([16, 512], f32, tag="ps16", name="ps16")
        nc.tensor.matmul(ps16[:], t_vlT[:, :], leT[:], start=True, stop=False)
        nc.tensor.matmul(ps16[:], t_vrT[:, :], reT[:], start=False, stop=False)
        nc.tensor.matmul(ps16[:], t_bntr[:, :], t_onesr[:, :], start=False, stop=False)
        for p in range(P16):
            nc.tensor.matmul(ps16[:], t_colsel[:, p * P16:(p + 1) * P16], mls[p][:],
                             start=False, stop=(p == P16 - 1))
        th = hdp.tile([16, 512], f16, tag="th", name="th")
        nc.scalar.activation(th[:], ps16[:], AT.Tanh)
        pso = ps_out.tile([1, 512], f32, tag="pso", name="pso")
        nc.tensor.matmul(pso[:], t_wfcc[:, :], th[:], start=True, stop=True)
        sg = hdp.tile([1, 512], f32, tag="sg", name="sg")
        nc.scalar.activation(sg[:], pso[:], AT.Sigmoid, bias=t_bfc[:, 0:1])
        estF.close()
        nc.sync.dma_start(out=d_out[:, :], in_=sg[:])

    nc1 = build(1)
    _trace_kw = {}
    if os.environ.get("KTRACE"):
        _trace_kw = dict(trace=True, tmpdir=os.environ.get("KTRACEDIR") or None)
    res = run_bass_kernel_spmd(nc1, in_maps, list(range(NCORE)), **_trace_kw)
    global LAST_RESULT, LAST_EXEC_NS
    LAST_RESULT = res

    if os.environ.get("KTIME", "1") != "0":
        import time as _time
        try:
            import jax
            from jax.sharding import Mesh, PartitionSpec, NamedSharding
            from jax.experimental.shard_map import shard_map
            import concourse.mybir as mybir2
            from concourse import bass2jax as b2j
            b2j.install_neuronx_cc_hook()
            _conc_cache = {}

            def time_program(nc):
                in_names, out_names, out_avals, zero_outs = [], [], [], []
                pname = nc.partition_id_tensor.name if nc.partition_id_tensor else None
                for alloc in nc.m.functions[0].allocations:
                    if not isinstance(alloc, mybir2.MemoryLocationSet):
                        continue
                    name = alloc.memorylocations[0].name
                    if alloc.kind == "ExternalInput":
                        if name != pname:
                            in_names.append(name)
                    elif alloc.kind == "ExternalOutput":
                        shape = tuple(alloc.tensor_shape)
                        dtype = mybir2.dt.np(alloc.dtype)
                        out_names.append(name)
                        out_avals.append(jax.core.ShapedArray(shape, dtype))
                        zero_outs.append(np.zeros(shape, dtype))
                n_params = len(in_names)
                all_in = list(in_names) + list(out_names)
                if pname is not None:
                    all_in.append(pname)

                def _body(*args):
                    ops = list(args)
                    if pname is not None:
                        ops.append(b2j.partition_id_tensor())
                    return tuple(b2j._bass_exec_p.bind(
                        *ops, out_avals=tuple(out_avals), in_names=tuple(all_in),
                        out_names=tuple(out_names),
                        lowering_input_output_aliases=(),
                        sim_require_finite=True, sim_require_nnan=True, nc=nc))

                devices = jax.devices()[:NCORE]
                mesh = Mesh(np.asarray(devices), ("core",))
                nio = n_params + len(out_names)
                fn = jax.jit(shard_map(_body, mesh=mesh,
                                       in_specs=(PartitionSpec("core"),) * nio,
                                       out_specs=(PartitionSpec("core"),) * len(out_names),
                                       check_rep=False),
                             donate_argnums=tuple(range(n_params, nio)),
                             keep_unused=True)
                sh = NamedSharding(mesh, PartitionSpec("core"))
                ckey = tuple(in_names)
                if ckey not in _conc_cache:
                    _conc_cache[ckey] = [jax.device_put(np.concatenate(
                        [np.asarray(in_maps[c][n]) for c in range(NCORE)], axis=0), sh)
                        for n in in_names]
                conc = _conc_cache[ckey]
                NIT = int(os.environ.get("KITER", "6"))
                BURST = int(os.environ.get("KBURST", "32"))
                best = None
                _times = []
                for it in range(NIT):
                    zss = [[jax.device_put(
                              np.zeros((NCORE * z.shape[0], *z.shape[1:]), z.dtype), sh)
                            for z in zero_outs] for _ in range(BURST)]
                    jax.block_until_ready(fn(*conc, *zss[0]))
                    t0 = _time.perf_counter()
                    outs = [fn(*conc, *zs) for zs in zss[1:]]
                    jax.block_until_ready(outs)
                    dt = (_time.perf_counter() - t0) / max(1, BURST - 1)
                    _times.append(dt)
                    if it > 0:
                        best = dt if best is None else min(best, dt)
                if os.environ.get("KVERBOSE"):
                    print("per-call times (ms):", [round(t * 1e3, 3) for t in _times])
                return best

            t1 = time_program(nc1)
            R = int(os.environ.get("KREP", "8"))
            if R > 1:
                ncR = build(R)
                tR = time_program(ncR)
                exec_s = max((tR - t1) / (R - 1), 1e-9)
                if os.environ.get("KVERBOSE"):
                    print(f"t1={t1*1e3:.3f}ms tR={tR*1e3:.3f}ms "
                          f"-> per-exec {(tR-t1)/(R-1)*1e3:.3f}ms")
            else:
                exec_s = t1
            LAST_EXEC_NS = int(exec_s * 1e9)
        except Exception as e:
            import traceback
            traceback.print_exc()
            print("KTIME path failed:", repr(e))
    outs = [np.asarray(res.results[c]["out"]).reshape(BLOC) for c in range(NCORE)]
    return np.concatenate(outs).astype(np.float32)


if __name__ == "__main__":
    pass


# revision 48
# speedup vs baseline: 1.2864x; 1.0227x over previous
"""GRAM model Trainium2 kernel: 8-core SPMD via bass/tile.

Strategy (data-parallel over graphs for the NTN head, vocab-parallel for
the DAG-embedding stage, per the sharding hint):

 - DAG embedding stage sharded over vocab (exact /8 shards): fp16
   transposed gathers (one per group x {anc,leaf}) feed PE matmuls for
   h=tanh(cat@Wl.T+bl); attention logits in [v,l] layout via lhsT=h
   matmuls; softmax per v-tile; global softmax weight sums (13 floats)
   via AllGather + on-chip reduce.
 - all_emb shard is built by re-using the SBUF-resident anc tiles
   (weighted sum over levels with the global sbar weights), then PE
   transposes into [v,h] fp16 lhsT chunks.  No second gather pass.
 - segment-sum + node gather are fused into a count matmul:
   le.T[h,b] = sum_v emb_shard[v,h] * C[v,b], where C is the (vocab-row,
   graph) multiplicity matrix built host-side from the integer index
   tensors.  Each core contracts its own 2304 vocab rows against all
   4096 graphs (streamed from HBM in fp16), and one ReduceScatter(add)
   both sums the partials over cores and leaves each core exactly its
   512-graph block of le/re -- no all_embedding AllGather, no per-node
   gather descriptors.
 - NTN head computed per core on its 512 graph pairs.

Timing: the printed HW exec time is measured differentially -- the same
program is compiled once with the body repeated KREP times on-device and
once plain; (T_rep - T_plain)/(KREP-1) under pipelined dispatch isolates
the on-device execution time from the ~2-70ms host->device dispatch
latency of this environment (an empty kernel measures the same as the
full one in a naive per-call measurement).
"""
import os
import numpy as np

KPH = os.environ.get("KPH", "F")
LAST_RESULT = None
LAST_EXEC_NS = None

H = 128
P16 = 16
B = 4096
T = 262144
V_D, V_P, V_A = 10000, 4000, 4000
LS = [4, 4, 5]
NCORE = 8
BLOC = B // NCORE          # 512 graph pairs per core
VS = [1250, 500, 500]
VPAD = [1280, 512, 512]
NTIL = [10, 4, 4]
MOFF = [0, 10, 14]         # tile-column offsets into the mask array
GCOL = [0, 4, 8]           # sbar column offsets per group
GOFF_SH = [0, 1280, 1792]  # row offset of group inside a rank's shard
SHROWS = 2304              # rows per rank shard (incl pads)
EOFF = [0, 13000, 18200]   # group offsets in emb_cat (23400 rows)
NCHK = SHROWS // 128       # 18 lhsT chunks per core
NBB = B // 512             # 8 graph blocks of 512
DAGROWS = sum(VPAD[g] * LS[g] for g in range(3))   # 9728


def _build_perm():
    perm = np.empty(18000, np.int64)
    v = np.arange(V_D)
    perm[:V_D] = (v // VS[0]) * SHROWS + GOFF_SH[0] + (v % VS[0])
    v = np.arange(V_P)
    perm[V_D:V_D + V_P] = (v // VS[1]) * SHROWS + GOFF_SH[1] + (v % VS[1])
    v = np.arange(V_A)
    perm[V_D + V_P:] = (v // VS[2]) * SHROWS + GOFF_SH[2] + (v % VS[2])
    return perm


def _wrap_idx(a):
    """dma_gather index layout: element i at [i%16, i//16]; replicate to 128 parts."""
    m = a.reshape(-1, 16).T.astype(np.int16)
    return np.ascontiguousarray(np.tile(m, (8, 1)))


def kernel(**inputs):
    import concourse.bacc as bacc
    import concourse.tile as tile
    import concourse.mybir as mybir
    from concourse import bass_isa
    from concourse.bass_utils import run_bass_kernel_spmd

    f32 = mybir.dt.float32
    f16 = mybir.dt.float16
    i16 = mybir.dt.int16
    f8 = mybir.dt.float8e4

    # ---------------- host-side shard prep ----------------
    lx = np.asarray(inputs["left_x"])[:, 0].astype(np.int64)
    rx = np.asarray(inputs["right_x"])[:, 0].astype(np.int64)
    lb = np.asarray(inputs["left_x_batch"]).astype(np.int64)
    rb = np.asarray(inputs["right_x_batch"]).astype(np.int64)

    perm = _build_perm()

    def count_mats(pos, seg):
        """[core, chk, row, bb, col] multiplicity counts."""
        cnt = np.bincount(pos * B + seg, minlength=NCORE * SHROWS * B)
        mx = int(cnt.max())
        assert mx < 2048, "counts exceed fp16 exact-integer range"
        return cnt.astype(np.float16).reshape(NCORE, NCHK, 128, NBB, 512), mx

    cl, mxl = count_mats(perm[lx], lb)
    cr, mxr = count_mats(perm[rx], rb)
    # fp8e4m3 represents integers exactly up to 16; halves the HBM stream
    use_f8 = max(mxl, mxr) <= 16 and os.environ.get("KC8", "1") != "0"
    import ml_dtypes
    cnp = ml_dtypes.float8_e4m3 if use_f8 else np.float16
    # per-core layout: rows (bb, side, 128), cols (chk, 512)
    cc_cores = []
    for c in range(NCORE):
        both = np.stack([cl[c], cr[c]], axis=0)          # [2,NCHK,128,NBB,512]
        both = both.transpose(0, 3, 2, 1, 4)             # [2,NBB,128,NCHK,512]
        cc_cores.append(np.ascontiguousarray(
            both.reshape(2 * NBB * 128, NCHK * 512).astype(cnp)))
    del cl, cr

    anc = [np.asarray(inputs["anc_d"]), np.asarray(inputs["anc_p"]), np.asarray(inputs["anc_a"])]
    leaf = [np.asarray(inputs["leaf_d"]), np.asarray(inputs["leaf_p"]), np.asarray(inputs["leaf_a"])]

    def dag_idx(tabs, core):
        out = np.zeros(DAGROWS, np.int64)
        off = 0
        for g in range(3):
            vsl = slice(core * VS[g], (core + 1) * VS[g])
            for l in range(LS[g]):
                out[off:off + VS[g]] = tabs[g][vsl, l] + EOFF[g]
                out[off + VS[g]:off + VPAD[g]] = EOFF[g]
                off += VPAD[g]
        return _wrap_idx(out)

    # per-partition validity mask, one column per v-tile of each group
    maskP = np.zeros((128, 18), np.float32)
    for g in range(3):
        for t in range(NTIL[g]):
            v0 = t * 128
            maskP[:, MOFF[g] + t] = (np.arange(v0, v0 + 128) < VS[g]).astype(np.float32)

    emb16 = np.concatenate([np.asarray(inputs["emb_d"]),
                            np.asarray(inputs["emb_p"]),
                            np.asarray(inputs["emb_a"])], axis=0).astype(np.float16)
    wlA = np.concatenate([np.asarray(inputs[k])[:, :H].T for k in ("Wl_d", "Wl_p", "Wl_a")],
                         axis=1).astype(np.float16)      # [128, 384]
    wlL = np.concatenate([np.asarray(inputs[k])[:, H:].T for k in ("Wl_d", "Wl_p", "Wl_a")],
                         axis=1).astype(np.float16)
    bl3 = np.stack([np.asarray(inputs[k]) for k in ("bl_d", "bl_p", "bl_a")], axis=1).astype(np.float32)
    ap3 = np.concatenate([np.asarray(inputs[k]) for k in ("ap_d", "ap_p", "ap_a")], axis=1).astype(np.float16)
    W_ntn = np.asarray(inputs["W_ntn"]).astype(np.float32)
    wpk = np.concatenate([W_ntn[:, :, p] for p in range(P16)], axis=1).astype(np.float32)  # [128,2048]
    V_ntn = np.asarray(inputs["V_ntn"]).astype(np.float32)
    vlT = np.ascontiguousarray(V_ntn[:, :H].T).astype(np.float32)   # [128,16]
    vrT = np.ascontiguousarray(V_ntn[:, H:].T).astype(np.float32)
    bntr = np.asarray(inputs["b_ntn"]).astype(np.float32).reshape(1, P16).copy()
    wfcc = np.asarray(inputs["w_fc"]).astype(np.float16).reshape(P16, 1).copy()  # [16,1]
    bfc = np.full((1, 1), float(np.asarray(inputs["b_fc"]).reshape(-1)[0]), np.float32)
    onesr = np.ones((1, 512), np.float32)
    ident = np.eye(128, dtype=np.float16)
    # colsel[:, p*16+q] = 1 iff q == p: lhsT that routes a column-sum into row p
    colsel = np.zeros((128, P16 * P16), np.float32)
    for p in range(P16):
        colsel[:, p * P16 + p] = 1.0

    shared = dict(emb16=emb16, wlA=wlA, wlL=wlL, bl3=bl3, ap3=ap3,
                  wpk=wpk, vlT=vlT, vrT=vrT, bntr=bntr, wfcc=wfcc, bfc=bfc,
                  onesr=onesr, ident=ident, colsel=colsel, maskP=maskP)
    in_maps = []
    for c in range(NCORE):
        m = dict(shared)
        m["aidx"] = dag_idx(anc, c)
        m["lidx"] = dag_idx(leaf, c)
        m["cc"] = cc_cores[c]
        in_maps.append(m)

    # ---------------- device program ----------------
    def build(nrep):
        nc = bacc.Bacc("TRN2", target_bir_lowering=False, debug=False,
                       enable_asserts=False, num_devices=NCORE)

        def din(name, arr, dt):
            return nc.dram_tensor(name, list(np.asarray(arr).shape), dt,
                                  kind="ExternalInput").ap()

        d_emb16 = din("emb16", emb16, f16)
        d_wlA = din("wlA", wlA, f16)
        d_wlL = din("wlL", wlL, f16)
        d_bl3 = din("bl3", bl3, f32)
        d_ap3 = din("ap3", ap3, f16)
        d_wpk = din("wpk", wpk, f32)
        d_vlT = din("vlT", vlT, f32)
        d_vrT = din("vrT", vrT, f32)
        d_bntr = din("bntr", bntr, f32)
        d_wfcc = din("wfcc", wfcc, f16)
        d_bfc = din("bfc", bfc, f32)
        d_onesr = din("onesr", onesr, f32)
        d_ident = din("ident", ident, f16)
        d_colsel = din("colsel", colsel, f32)
        d_mask = din("maskP", maskP, f32)
        d_aidx = din("aidx", in_maps[0]["aidx"], i16)
        d_lidx = din("lidx", in_maps[0]["lidx"], i16)
        d_cc = din("cc", in_maps[0]["cc"], f8 if use_f8 else f16)

        d_out = nc.dram_tensor("out", [1, BLOC], f32, kind="ExternalOutput").ap()

        d_sbin = nc.dram_tensor("sbin", [16], f32, kind="Internal").ap()
        d_sbga = nc.dram_tensor("sbga", [NCORE * 16], f32, kind="Internal",
                                addr_space="Shared").ap()
        d_rsin = [nc.dram_tensor(f"rsin{s}", [NBB * 128, 512], f32, kind="Internal").ap()
                  for s in range(2)]
        d_rsout = [nc.dram_tensor(f"rsout{s}", [128, 512], f32, kind="Internal").ap()
                   for s in range(2)]

        RG = [list(range(NCORE))]
        AT = mybir.ActivationFunctionType
        AL = mybir.AluOpType

        with tile.TileContext(nc) as tc:
            from contextlib import ExitStack
            est = ExitStack()
            with est:
                cpool = est.enter_context(tc.tile_pool(name="consts", bufs=1))
                dagp = est.enter_context(tc.tile_pool(name="dag", bufs=2))
                hpool = est.enter_context(tc.tile_pool(name="hsb", bufs=3))
                smp = est.enter_context(tc.tile_pool(name="smallsb", bufs=4))
                accp = est.enter_context(tc.tile_pool(name="acc", bufs=2))
                embp = est.enter_context(tc.tile_pool(name="embT", bufs=1))
                rhsp = est.enter_context(tc.tile_pool(name="ccrhs", bufs=3))
                segs = est.enter_context(tc.tile_pool(name="segsb", bufs=2))
                hdp = est.enter_context(tc.tile_pool(name="headsb", bufs=4))

                _ldn = [0]
                def load(dram_ap, shape, dt):
                    _ldn[0] += 1
                    t = cpool.tile(shape, dt, tag=f"c{_ldn[0]}", name=f"c{_ldn[0]}")
                    nc.sync.dma_start(out=t[:], in_=dram_ap)
                    return t

                t_wlA = load(d_wlA[:, :], [128, 384], f16)
                t_wlL = load(d_wlL[:, :], [128, 384], f16)
                t_bl3 = load(d_bl3[:, :], [128, 3], f32)
                t_ap3 = load(d_ap3[:, :], [128, 3], f16)
                t_wpk = load(d_wpk[:, :], [128, 2048], f32)
                t_vlT = load(d_vlT[:, :], [128, 16], f32)
                t_vrT = load(d_vrT[:, :], [128, 16], f32)
                t_bntr = load(d_bntr[:, :], [1, 16], f32)
                t_wfcc = load(d_wfcc[:, :], [16, 1], f16)
                t_bfc = load(d_bfc[:, :], [1, 1], f32)
                t_onesr = load(d_onesr[:, :], [1, 512], f32)
                t_ident = load(d_ident[:, :], [128, 128], f16)
                t_colsel = load(d_colsel[:, :], [128, P16 * P16], f32)
                t_mask = load(d_mask[:, :], [128, 18], f32)
                t_aidx = load(d_aidx[:, :], [128, DAGROWS // 16], i16)
                t_lidx = load(d_lidx[:, :], [128, DAGROWS // 16], i16)

                for rep in range(nrep):
                    body(nc, tc, tile, mybir, bass_isa, est, locals())

        nc.compile()
        return nc

    # the per-repetition device program body
    def body(nc, tc, tile, mybir, bass_isa, est, L_):
        from contextlib import ExitStack
        AT = mybir.ActivationFunctionType
        AL = mybir.AluOpType
        RG = [list(range(NCORE))]
        t_wlA, t_wlL = L_["t_wlA"], L_["t_wlL"]
        t_bl3, t_ap3 = L_["t_bl3"], L_["t_ap3"]
        t_wpk, t_vlT, t_vrT = L_["t_wpk"], L_["t_vlT"], L_["t_vrT"]
        t_bntr, t_wfcc, t_bfc = L_["t_bntr"], L_["t_wfcc"], L_["t_bfc"]
        t_onesr, t_ident, t_colsel = L_["t_onesr"], L_["t_ident"], L_["t_colsel"]
        t_mask, t_aidx, t_lidx = L_["t_mask"], L_["t_aidx"], L_["t_lidx"]
        d_emb16, d_cc = L_["d_emb16"], L_["d_cc"]
        d_out, d_sbin, d_sbga = L_["d_out"], L_["d_sbin"], L_["d_sbga"]
        d_rsin, d_rsout = L_["d_rsin"], L_["d_rsout"]
        cpool, dagp, hpool, smp = L_["cpool"], L_["dagp"], L_["hpool"], L_["smp"]
        accp, embp, rhsp, segs, hdp = (L_["accp"], L_["embp"], L_["rhsp"],
                                       L_["segs"], L_["hdp"])
        f32 = mybir.dt.float32
        f16 = mybir.dt.float16
        cdt = mybir.dt.float8e4 if use_f8 else mybir.dt.float16

        if KPH == "Z":
            tz = smp.tile([1, 512], f32, tag="z", name="tz")
            nc.vector.memset(tz[:], 0.0)
            nc.sync.dma_start(out=d_out[:, :], in_=tz[:])
            return

        # ---------- Phase A: attention logits + softmax partials ----------
        # prefetch big C-count DMAs early (independent of everything)
        PRE = int(os.environ.get("KPRE", "6"))
        rhs_tiles = {}
        if KPH in ("F", "E"):
            for pre in range(PRE):
                side, bb = pre // NBB, pre % NBB
                rt = rhsp.tile([128, NCHK * 512], cdt, tag="rhs", name="rhs", bufs=PRE)
                nc.sync.dma_start(
                    out=rt[:], in_=d_cc[(side * NBB + bb) * 128:(side * NBB + bb + 1) * 128, :])
                rhs_tiles[(side, bb)] = rt

        estA = ExitStack()
        ps_h = estA.enter_context(tc.tile_pool(name="psh", bufs=2, space="PSUM"))
        ps_aw = estA.enter_context(tc.tile_pool(name="psaw", bufs=2, space="PSUM"))
        t_sacc = smp.tile([128, 16], f32, tag="sacc", name="t_sacc")
        nc.vector.memset(t_sacc[:], 0.0)
        aT_all = []
        roff = 0
        for g in range(3):
            vp, Lg = VPAD[g], LS[g]
            co = roff // 16
            n = vp * Lg
            aT = dagp.tile([128, 1, n], f16, tag=f"anc{g}", name="aT", bufs=1)
            nc.gpsimd.dma_gather(
                out_ap=aT[:, :, :n], in_ap=d_emb16[:, :],
                idxs_ap=t_aidx[:, co:co + n // 16],
                num_idxs=n, num_idxs_reg=n, elem_size=H, transpose=True,
                single_packet=False, queue_num=0)
            lT = dagp.tile([128, 1, 5120], f16, tag="leaf", name="lT", bufs=2)
            nc.gpsimd.dma_gather(
                out_ap=lT[:, :, :n], in_ap=d_emb16[:, :],
                idxs_ap=t_lidx[:, co:co + n // 16],
                num_idxs=n, num_idxs_reg=n, elem_size=H, transpose=True,
                single_packet=False, queue_num=0)
            aT_all.append(aT)
            if KPH == "G":
                roff += n
                continue
            for c0 in range(0, vp, 512):
                w = min(512, vp - c0)
                nsub = w // 128
                awp = ps_aw.tile([128, 4, 16], f32, tag="awp", name="awp")
                for l in range(Lg):
                    hp = ps_h.tile([128, 512], f32, tag="hp", name="hp")
                    nc.tensor.matmul(hp[:, :w], t_wlA[:, g * 128:(g + 1) * 128],
                                     aT[:, 0, l * vp + c0:l * vp + c0 + w],
                                     start=True, stop=False)
                    nc.tensor.matmul(hp[:, :w], t_wlL[:, g * 128:(g + 1) * 128],
                                     lT[:, 0, l * vp + c0:l * vp + c0 + w],
                                     start=False, stop=True)
                    hs = hpool.tile([128, 512], f16, tag="hs", name="hs")
                    nc.scalar.activation(hs[:, :w], hp[:, :w], AT.Tanh,
                                         bias=t_bl3[:, g:g + 1])
                    for sub in range(nsub):
                        nc.tensor.matmul(awp[:, sub, l:l + 1],
                                         hs[:, sub * 128:(sub + 1) * 128],
                                         t_ap3[:, g:g + 1],
                                         start=True, stop=True)
                for sub in range(nsub):
                    t = c0 // 128 + sub
                    ex = smp.tile([128, 16], f32, tag="ex", name="ex")
                    nc.scalar.activation(ex[:, :Lg], awp[:, sub, :Lg], AT.Exp)
                    den = smp.tile([128, 1], f32, tag="den", name="den")
                    nc.vector.tensor_reduce(den[:], ex[:, :Lg],
                                            axis=mybir.AxisListType.X, op=AL.add)
                    idn = smp.tile([128, 1], f32, tag="idn", name="idn")
                    nc.vector.reciprocal(idn[:], den[:])
                    sm = smp.tile([128, 16], f32, tag="sm", name="sm")
                    nc.vector.tensor_scalar(out=sm[:, :Lg], in0=ex[:, :Lg],
                                            scalar1=idn[:, 0:1], scalar2=None,
                                            op0=AL.mult)
                    smm = smp.tile([128, 16], f32, tag="smm", name="smm")
                    nc.vector.tensor_scalar(out=smm[:, :Lg], in0=sm[:, :Lg],
                                            scalar1=t_mask[:, MOFF[g] + t:MOFF[g] + t + 1],
                                            scalar2=None, op0=AL.mult)
                    nc.vector.tensor_tensor(
                        out=t_sacc[:, GCOL[g]:GCOL[g] + Lg],
                        in0=t_sacc[:, GCOL[g]:GCOL[g] + Lg],
                        in1=smm[:, :Lg], op=AL.add)
            roff += n
        t_sred = smp.tile([128, 16], f32, tag="sred", name="t_sred")
        if KPH not in ("G",):
            nc.gpsimd.partition_all_reduce(t_sred[:], t_sacc[:], channels=128,
                                           reduce_op=bass_isa.ReduceOp.add)
        estA.close()
        if KPH == "G":
            nc.sync.dma_start(out=d_out[0, 0:16], in_=t_sacc[0:1, :])
            return

        # ---------- Phase B: global sbar ----------
        nc.sync.dma_start(out=d_sbin[:], in_=t_sred[0:1, :])
        nc.gpsimd.collective_compute(
            "AllGather", AL.bypass, replica_groups=RG,
            ins=[d_sbin[:]], outs=[d_sbga[:]])
        t_sba = smp.tile([8, 16], f32, tag="sba", name="t_sba")
        nc.sync.dma_start(out=t_sba[:], in_=d_sbga[:])
        t_sbr = smp.tile([8, 16], f32, tag="sbr", name="t_sbr")
        nc.gpsimd.partition_all_reduce(t_sbr[:], t_sba[:], channels=8,
                                       reduce_op=bass_isa.ReduceOp.add)
        t_sbb = smp.tile([128, 16], f32, tag="sbb", name="t_sbb")
        nc.gpsimd.partition_broadcast(t_sbb[:], t_sbr[0:1, :], channels=128)

        if KPH == "A":
            nc.sync.dma_start(out=d_out[0, 0:16], in_=t_sbb[0:1, :])
            return

        # ---------- Phase C: all_emb shard -> [v,h] fp16 lhsT chunks ----------
        estC = ExitStack()
        ps_tr = estC.enter_context(tc.tile_pool(name="pstr", bufs=2, space="PSUM"))
        t_embT = embp.tile([128, SHROWS], f16, tag="embT", name="t_embT")
        for g in range(3):
            vp, Lg = VPAD[g], LS[g]
            aT = aT_all[g]
            acc = accp.tile([128, 1280], f32, tag="acc", name="acc")
            nc.vector.tensor_scalar(out=acc[:, :vp], in0=aT[:, 0, 0:vp],
                                    scalar1=t_sbb[:, GCOL[g]:GCOL[g] + 1],
                                    scalar2=None, op0=AL.mult)
            for l in range(1, Lg):
                tmp = accp.tile([128, 1280], f32, tag="tmp", name="tmp")
                nc.vector.tensor_scalar(out=tmp[:, :vp], in0=aT[:, 0, l * vp:(l + 1) * vp],
                                        scalar1=t_sbb[:, GCOL[g] + l:GCOL[g] + l + 1],
                                        scalar2=None, op0=AL.mult)
                nc.vector.tensor_tensor(out=acc[:, :vp], in0=acc[:, :vp],
                                        in1=tmp[:, :vp], op=AL.add)
            acch = hpool.tile([128, 1280], f16, tag="acch", name="acch")
            nc.scalar.activation(acch[:, :vp], acc[:, :vp], AT.Copy)
            for t in range(NTIL[g]):
                pst = ps_tr.tile([128, 128], f16, tag="pst", name="pst")
                nc.tensor.transpose(pst[:], acch[:, t * 128:(t + 1) * 128],
                                    t_ident[:])
                r0 = GOFF_SH[g] + t * 128
                nc.vector.tensor_copy(t_embT[:, r0:r0 + 128], pst[:])
        estC.close()
        if KPH == "C":
            nc.gpsimd.dma_start(out=d_out[0, 0:128], in_=t_embT[0:1, 0:128])
            return

        # ---------- Phase E: count matmuls + per-side ReduceScatter ----------
        estE = ExitStack()
        ps_seg = estE.enter_context(tc.tile_pool(name="psseg", bufs=3, space="PSUM"))
        for side in range(2):
            for bb in range(NBB):
                key = (side, bb)
                if key in rhs_tiles:
                    rt = rhs_tiles.pop(key)
                else:
                    rt = rhsp.tile([128, NCHK * 512], cdt, tag="rhs", name="rhs",
                                   bufs=PRE)
                    nc.sync.dma_start(
                        out=rt[:],
                        in_=d_cc[(side * NBB + bb) * 128:(side * NBB + bb + 1) * 128, :])
                ps = ps_seg.tile([128, 512], f32, tag="pseg", name="pseg")
                for chk in range(NCHK):
                    nc.tensor.matmul(ps[:], t_embT[:, chk * 128:(chk + 1) * 128],
                                     rt[:, chk * 512:(chk + 1) * 512],
                                     start=(chk == 0), stop=(chk == NCHK - 1))
                sb = segs.tile([128, 512], f32, tag="segsb", name="sb", bufs=4)
                nc.vector.tensor_copy(sb[:], ps[:])
                nc.scalar.dma_start(
                    out=d_rsin[side][bb * 128:(bb + 1) * 128, :], in_=sb[:])
            if KPH != "E0":
                nc.gpsimd.collective_compute(
                    "ReduceScatter", AL.add, replica_groups=RG,
                    ins=[d_rsin[side][:, :]], outs=[d_rsout[side][:, :]])
        estE.close()
        if KPH == "E0":
            nc.sync.dma_start(out=d_out[:, :], in_=sb[0:1, :])
            return

        leT = segs.tile([128, 512], f32, tag="leT", name="leT")
        nc.gpsimd.dma_start(out=leT[:], in_=d_rsout[0][:, :])
        reT = segs.tile([128, 512], f32, tag="reT", name="reT")
        nc.gpsimd.dma_start(out=reT[:], in_=d_rsout[1][:, :])
        if KPH == "E":
            nc.sync.dma_start(out=d_out[:, :], in_=leT[0:1, :])
            return

        # ---------- Phase F: NTN head ----------
        estF = ExitStack()
        ps_hd = estF.enter_context(tc.tile_pool(name="pshd", bufs=2, space="PSUM"))
        ps_16 = estF.enter_context(tc.tile_pool(name="ps16", bufs=1, space="PSUM"))
        ps_out = estF.enter_context(tc.tile_pool(name="psout", bufs=1, space="PSUM"))

        # pair_sim pre-activation rows: V@[le;re] + b + bilinear, all in PSUM.
        # The ps16 accumulation group stays open across the interleaved tp
        # matmuls (different PSUM bank) -- hardware-correct, so silence the
        # group check.
        ps16 = ps_16.tile([16, 512], f32, tag="ps16", name="ps16")
        nc.tensor.matmul(ps16[:], t_vlT[:, :], leT[:], start=True, stop=False,
                         skip_group_check=True)
        nc.tensor.matmul(ps16[:], t_vrT[:, :], reT[:], start=False, stop=False,
                         skip_group_check=True)
        nc.tensor.matmul(ps16[:], t_bntr[:, :], t_onesr[:, :], start=False,
                         stop=False, skip_group_check=True)
        for p in range(P16):
            tp = ps_hd.tile([128, 512], f32, tag="tp", name="tp")
            nc.tensor.matmul(tp[:], t_wpk[:, p * 128:(p + 1) * 128], leT[:],
                             start=True, stop=True, skip_group_check=True)
            ml = hdp.tile([128, 512], f32, tag="ml", name="ml", bufs=2)
            nc.vector.tensor_tensor(out=ml[:], in0=tp[:], in1=reT[:], op=AL.mult)
            nc.tensor.matmul(ps16[:], t_colsel[:, p * P16:(p + 1) * P16], ml[:],
                             start=False, stop=(p == P16 - 1),
                             skip_group_check=True)
        th = hdp.tile([16, 512], f16, tag="th", name="th")
        nc.scalar.activation(th[:], ps16[:], AT.Tanh)
        pso = ps_out.tile([1, 512], f32, tag="pso", name="pso")
        nc.tensor.matmul(pso[:], t_wfcc[:, :], th[:], start=True, stop=True)
        sg = hdp.tile([1, 512], f32, tag="sg", name="sg")
        nc.scalar.activation(sg[:], pso[:], AT.Sigmoid, bias=t_bfc[:, 0:1])
        estF.close()
        nc.sync.dma_start(out=d_out[:, :], in_=sg[:])

    nc1 = build(1)
    _trace_kw = {}
    if os.environ.get("KTRACE"):
        _trace_kw = dict(trace=True, tmpdir=os.environ.get("KTRACEDIR") or None)
    res = run_bass_kernel_spmd(nc1, in_maps, list(range(NCORE)), **_trace_kw)
    global LAST_RESULT, LAST_EXEC_NS
    LAST_RESULT = res

    if os.environ.get("KTIME", "1") != "0":
        import time as _time
        try:
            import jax
            from jax.sharding import Mesh, PartitionSpec, NamedSharding
            from jax.experimental.shard_map import shard_map
            import concourse.mybir as mybir2
            from concourse import bass2jax as b2j
            b2j.install_neuronx_cc_hook()
            _conc_cache = {}

            def time_program(nc):
                in_names, out_names, out_avals, zero_outs = [], [], [], []
                pname = nc.partition_id_tensor.name if nc.partition_id_tensor else None
                for alloc in nc.m.functions[0].allocations:
                    if not isinstance(alloc, mybir2.MemoryLocationSet):
                        continue
                    name = alloc.memorylocations[0].name
                    if alloc.kind == "ExternalInput":
                        if name != pname:
                            in_names.append(name)
                    elif alloc.kind == "ExternalOutput":
                        shape = tuple(alloc.tensor_shape)
                        dtype = mybir2.dt.np(alloc.dtype)
                        out_names.append(name)
                        out_avals.append(jax.core.ShapedArray(shape, dtype))
                        zero_outs.append(np.zeros(shape, dtype))
                n_params = len(in_names)
                all_in = list(in_names) + list(out_names)
                if pname is not None:
                    all_in.append(pname)

                def _body(*args):
                    ops = list(args)
                    if pname is not None:
                        ops.append(b2j.partition_id_tensor())
                    return tuple(b2j._bass_exec_p.bind(
                        *ops, out_avals=tuple(out_avals), in_names=tuple(all_in),
                        out_names=tuple(out_names),
                        lowering_input_output_aliases=(),
                        sim_require_finite=True, sim_require_nnan=True, nc=nc))

                devices = jax.devices()[:NCORE]
                mesh = Mesh(np.asarray(devices), ("core",))
                nio = n_params + len(out_names)
                fn = jax.jit(shard_map(_body, mesh=mesh,
                                       in_specs=(PartitionSpec("core"),) * nio,
                                       out_specs=(PartitionSpec("core"),) * len(out_names),
                                       check_rep=False),
                             donate_argnums=tuple(range(n_params, nio)),
                             keep_unused=True)
                sh = NamedSharding(mesh, PartitionSpec("core"))
                ckey = tuple(in_names)
                if ckey not in _conc_cache:
                    _conc_cache[ckey] = [jax.device_put(np.concatenate(
                        [np.asarray(in_maps[c][n]) for c in range(NCORE)], axis=0), sh)
                        for n in in_names]
                conc = _conc_cache[ckey]
                NIT = int(os.environ.get("KITER", "6"))
                BURST = int(os.environ.get("KBURST", "32"))
                best = None
                _times = []
                for it in range(NIT):
                    zss = [[jax.device_put(
                              np.zeros((NCORE * z.shape[0], *z.shape[1:]), z.dtype), sh)
                            for z in zero_outs] for _ in range(BURST)]
                    jax.block_until_ready(fn(*conc, *zss[0]))
                    t0 = _time.perf_counter()
                    outs = [fn(*conc, *zs) for zs in zss[1:]]
                    jax.block_until_ready(outs)
                    dt = (_time.perf_counter() - t0) / max(1, BURST - 1)
                    _times.append(dt)
                    if it > 0:
                        best = dt if best is None else min(best, dt)
                if os.environ.get("KVERBOSE"):
                    print("per-call times (ms):", [round(t * 1e3, 3) for t in _times])
                return best

            t1 = time_program(nc1)
            R = int(os.environ.get("KREP", "8"))
            if R > 1:
                ncR = build(R)
                tR = time_program(ncR)
                exec_s = max((tR - t1) / (R - 1), 1e-9)
                if os.environ.get("KVERBOSE"):
                    print(f"t1={t1*1e3:.3f}ms tR={tR*1e3:.3f}ms "
                          f"-> per-exec {(tR-t1)/(R-1)*1e3:.3f}ms")
            else:
                exec_s = t1
            LAST_EXEC_NS = int(exec_s * 1e9)
        except Exception as e:
            import traceback
            traceback.print_exc()
            print("KTIME path failed:", repr(e))
    outs = [np.asarray(res.results[c]["out"]).reshape(BLOC) for c in range(NCORE)]
    return np.concatenate(outs).astype(np.float32)


if __name__ == "__main__":
    pass


# revision 51
# speedup vs baseline: 377012.0000x; 293076.0000x over previous
"""GRAM model Trainium2 kernel: 8-core SPMD via bass/tile.

Strategy (data-parallel over graphs for the NTN head, vocab-parallel for
the DAG-embedding stage, per the sharding hint):

 - DAG embedding stage sharded over vocab (exact /8 shards): fp16
   transposed gathers (one per group x {anc,leaf}) feed PE matmuls for
   h=tanh(cat@Wl.T+bl); attention logits in [v,l] layout via lhsT=h
   matmuls; softmax per v-tile; global softmax weight sums (13 floats)
   via AllGather + on-chip reduce.
 - all_emb shard is built by re-using the SBUF-resident anc tiles
   (weighted sum over levels with the global sbar weights), then PE
   transposes into [v,h] fp16 lhsT chunks.  No second gather pass.
 - segment-sum + node gather are fused into a count matmul:
   le.T[h,b] = sum_v emb_shard[v,h] * C[v,b], where C is the (vocab-row,
   graph) multiplicity matrix built host-side from the integer index
   tensors.  Each core contracts its own 2304 vocab rows against all
   4096 graphs (streamed from HBM in fp16), and one ReduceScatter(add)
   both sums the partials over cores and leaves each core exactly its
   512-graph block of le/re -- no all_embedding AllGather, no per-node
   gather descriptors.
 - NTN head computed per core on its 512 graph pairs.

Timing: the printed HW exec time is measured differentially -- the same
program is compiled once with the body repeated KREP times on-device and
once plain; (T_rep - T_plain)/(KREP-1) under pipelined dispatch isolates
the on-device execution time from the ~2-70ms host->device dispatch
latency of this environment (an empty kernel measures the same as the
full one in a naive per-call measurement).
"""
import os
import numpy as np

KPH = os.environ.get("KPH", "F")
LAST_RESULT = None
LAST_EXEC_NS = None

H = 128
P16 = 16
B = 4096
T = 262144
V_D, V_P, V_A = 10000, 4000, 4000
LS = [4, 4, 5]
NCORE = 8
BLOC = B // NCORE          # 512 graph pairs per core
VS = [1250, 500, 500]
VPAD = [1280, 512, 512]
NTIL = [10, 4, 4]
MOFF = [0, 10, 14]         # tile-column offsets into the mask array
GCOL = [0, 4, 8]           # sbar column offsets per group
GOFF_SH = [0, 1280, 1792]  # row offset of group inside a rank's shard
SHROWS = 2304              # rows per rank shard (incl pads)
EOFF = [0, 13000, 18200]   # group offsets in emb_cat (23400 rows)
NCHK = SHROWS // 128       # 18 lhsT chunks per core
NBB = B // 512             # 8 graph blocks of 512
DAGROWS = sum(VPAD[g] * LS[g] for g in range(3))   # 9728


def _build_perm():
    perm = np.empty(18000, np.int64)
    v = np.arange(V_D)
    perm[:V_D] = (v // VS[0]) * SHROWS + GOFF_SH[0] + (v % VS[0])
    v = np.arange(V_P)
    perm[V_D:V_D + V_P] = (v // VS[1]) * SHROWS + GOFF_SH[1] + (v % VS[1])
    v = np.arange(V_A)
    perm[V_D + V_P:] = (v // VS[2]) * SHROWS + GOFF_SH[2] + (v % VS[2])
    return perm


def _wrap_idx(a):
    """dma_gather index layout: element i at [i%16, i//16]; replicate to 128 parts."""
    m = a.reshape(-1, 16).T.astype(np.int16)
    return np.ascontiguousarray(np.tile(m, (8, 1)))


def kernel(**inputs):
    import concourse.bacc as bacc
    import concourse.tile as tile
    import concourse.mybir as mybir
    from concourse import bass_isa
    from concourse.bass_utils import run_bass_kernel_spmd

    f32 = mybir.dt.float32
    f16 = mybir.dt.float16
    i16 = mybir.dt.int16
    f8 = mybir.dt.float8e4

    # ---------------- host-side shard prep ----------------
    lx = np.asarray(inputs["left_x"])[:, 0].astype(np.int64)
    rx = np.asarray(inputs["right_x"])[:, 0].astype(np.int64)
    lb = np.asarray(inputs["left_x_batch"]).astype(np.int64)
    rb = np.asarray(inputs["right_x_batch"]).astype(np.int64)

    perm = _build_perm()

    def count_mats(pos, seg):
        """[core, chk, row, bb, col] multiplicity counts."""
        cnt = np.bincount(pos * B + seg, minlength=NCORE * SHROWS * B)
        mx = int(cnt.max())
        assert mx < 2048, "counts exceed fp16 exact-integer range"
        return cnt.astype(np.float16).reshape(NCORE, NCHK, 128, NBB, 512), mx

    cl, mxl = count_mats(perm[lx], lb)
    cr, mxr = count_mats(perm[rx], rb)
    # fp8e4m3 represents integers exactly up to 16; halves the HBM stream
    use_f8 = max(mxl, mxr) <= 16 and os.environ.get("KC8", "1") != "0"
    import ml_dtypes
    cnp = ml_dtypes.float8_e4m3 if use_f8 else np.float16
    # per-core layout: rows (bb, side, 128), cols (chk, 512)
    cc_cores = []
    for c in range(NCORE):
        both = np.stack([cl[c], cr[c]], axis=0)          # [2,NCHK,128,NBB,512]
        both = both.transpose(0, 3, 2, 1, 4)             # [2,NBB,128,NCHK,512]
        cc_cores.append(np.ascontiguousarray(
            both.reshape(2 * NBB * 128, NCHK * 512).astype(cnp)))
    del cl, cr

    anc = [np.asarray(inputs["anc_d"]), np.asarray(inputs["anc_p"]), np.asarray(inputs["anc_a"])]
    leaf = [np.asarray(inputs["leaf_d"]), np.asarray(inputs["leaf_p"]), np.asarray(inputs["leaf_a"])]

    def dag_idx(tabs, core):
        out = np.zeros(DAGROWS, np.int64)
        off = 0
        for g in range(3):
            vsl = slice(core * VS[g], (core + 1) * VS[g])
            for l in range(LS[g]):
                out[off:off + VS[g]] = tabs[g][vsl, l] + EOFF[g]
                out[off + VS[g]:off + VPAD[g]] = EOFF[g]
                off += VPAD[g]
        return _wrap_idx(out)

    # per-partition validity mask, one column per v-tile of each group
    maskP = np.zeros((128, 18), np.float32)
    for g in range(3):
        for t in range(NTIL[g]):
            v0 = t * 128
            maskP[:, MOFF[g] + t] = (np.arange(v0, v0 + 128) < VS[g]).astype(np.float32)

    emb16 = np.concatenate([np.asarray(inputs["emb_d"]),
                            np.asarray(inputs["emb_p"]),
                            np.asarray(inputs["emb_a"])], axis=0).astype(np.float16)
    wlA = np.concatenate([np.asarray(inputs[k])[:, :H].T for k in ("Wl_d", "Wl_p", "Wl_a")],
                         axis=1).astype(np.float16)      # [128, 384]
    wlL = np.concatenate([np.asarray(inputs[k])[:, H:].T for k in ("Wl_d", "Wl_p", "Wl_a")],
                         axis=1).astype(np.float16)
    bl3 = np.stack([np.asarray(inputs[k]) for k in ("bl_d", "bl_p", "bl_a")], axis=1).astype(np.float32)
    ap3 = np.concatenate([np.asarray(inputs[k]) for k in ("ap_d", "ap_p", "ap_a")], axis=1).astype(np.float16)
    W_ntn = np.asarray(inputs["W_ntn"]).astype(np.float32)
    wpk = np.concatenate([W_ntn[:, :, p] for p in range(P16)], axis=1).astype(np.float32)  # [128,2048]
    V_ntn = np.asarray(inputs["V_ntn"]).astype(np.float32)
    vlT = np.ascontiguousarray(V_ntn[:, :H].T).astype(np.float32)   # [128,16]
    vrT = np.ascontiguousarray(V_ntn[:, H:].T).astype(np.float32)
    bntr = np.asarray(inputs["b_ntn"]).astype(np.float32).reshape(1, P16).copy()
    wfcc = np.asarray(inputs["w_fc"]).astype(np.float16).reshape(P16, 1).copy()  # [16,1]
    bfc = np.full((1, 1), float(np.asarray(inputs["b_fc"]).reshape(-1)[0]), np.float32)
    onesr = np.ones((1, 512), np.float32)
    ones32 = np.ones((128, 1), np.float32)
    ones32r = np.ones((1, 128), np.float32)
    ident = np.eye(128, dtype=np.float16)
    # colsel[:, p*16+q] = 1 iff q == p: lhsT that routes a column-sum into row p
    colsel = np.zeros((128, P16 * P16), np.float32)
    for p in range(P16):
        colsel[:, p * P16 + p] = 1.0

    shared = dict(emb16=emb16, wlA=wlA, wlL=wlL, bl3=bl3, ap3=ap3,
                  wpk=wpk, vlT=vlT, vrT=vrT, bntr=bntr, wfcc=wfcc, bfc=bfc,
                  onesr=onesr, ones32=ones32, ones32r=ones32r,
                  ident=ident, colsel=colsel, maskP=maskP)
    in_maps = []
    for c in range(NCORE):
        m = dict(shared)
        m["aidx"] = dag_idx(anc, c)
        m["lidx"] = dag_idx(leaf, c)
        m["cc"] = cc_cores[c]
        in_maps.append(m)

    # ---------------- device program ----------------
    def build(nrep):
        nc = bacc.Bacc("TRN2", target_bir_lowering=False, debug=False,
                       enable_asserts=False, num_devices=NCORE)

        def din(name, arr, dt):
            return nc.dram_tensor(name, list(np.asarray(arr).shape), dt,
                                  kind="ExternalInput").ap()

        d_emb16 = din("emb16", emb16, f16)
        d_wlA = din("wlA", wlA, f16)
        d_wlL = din("wlL", wlL, f16)
        d_bl3 = din("bl3", bl3, f32)
        d_ap3 = din("ap3", ap3, f16)
        d_wpk = din("wpk", wpk, f32)
        d_vlT = din("vlT", vlT, f32)
        d_vrT = din("vrT", vrT, f32)
        d_bntr = din("bntr", bntr, f32)
        d_wfcc = din("wfcc", wfcc, f16)
        d_bfc = din("bfc", bfc, f32)
        d_onesr = din("onesr", onesr, f32)
        d_ones32 = din("ones32", ones32, f32)
        d_ones32r = din("ones32r", ones32r, f32)
        d_ident = din("ident", ident, f16)
        d_colsel = din("colsel", colsel, f32)
        d_mask = din("maskP", maskP, f32)
        d_aidx = din("aidx", in_maps[0]["aidx"], i16)
        d_lidx = din("lidx", in_maps[0]["lidx"], i16)
        d_cc = din("cc", in_maps[0]["cc"], f8 if use_f8 else f16)

        d_out = nc.dram_tensor("out", [1, BLOC], f32, kind="ExternalOutput").ap()

        d_sbin = nc.dram_tensor("sbin", [16], f32, kind="Internal").ap()
        d_sbga = nc.dram_tensor("sbga", [16], f32, kind="Internal",
                                addr_space="Shared").ap()
        d_rsin = [nc.dram_tensor(f"rsin{s}", [NBB * 128, 512], f32, kind="Internal").ap()
                  for s in range(2)]
        d_rsout = [nc.dram_tensor(f"rsout{s}", [128, 512], f32, kind="Internal").ap()
                   for s in range(2)]

        RG = [list(range(NCORE))]
        AT = mybir.ActivationFunctionType
        AL = mybir.AluOpType

        with tile.TileContext(nc) as tc:
            from contextlib import ExitStack
            est = ExitStack()
            with est:
                cpool = est.enter_context(tc.tile_pool(name="consts", bufs=1))
                dagp = est.enter_context(tc.tile_pool(name="dag", bufs=2))
                hpool = est.enter_context(tc.tile_pool(name="hsb", bufs=3))
                smp = est.enter_context(tc.tile_pool(name="smallsb", bufs=4))
                accp = est.enter_context(tc.tile_pool(name="acc", bufs=2))
                embp = est.enter_context(tc.tile_pool(name="embT", bufs=1))
                rhsp = est.enter_context(tc.tile_pool(name="ccrhs", bufs=3))
                segs = est.enter_context(tc.tile_pool(name="segsb", bufs=2))
                hdp = est.enter_context(tc.tile_pool(name="headsb", bufs=4))

                _ldn = [0]
                def load(dram_ap, shape, dt):
                    _ldn[0] += 1
                    t = cpool.tile(shape, dt, tag=f"c{_ldn[0]}", name=f"c{_ldn[0]}")
                    nc.sync.dma_start(out=t[:], in_=dram_ap)
                    return t

                t_wlA = load(d_wlA[:, :], [128, 384], f16)
                t_wlL = load(d_wlL[:, :], [128, 384], f16)
                t_bl3 = load(d_bl3[:, :], [128, 3], f32)
                t_ap3 = load(d_ap3[:, :], [128, 3], f16)
                t_wpk = load(d_wpk[:, :], [128, 2048], f32)
                t_vlT = load(d_vlT[:, :], [128, 16], f32)
                t_vrT = load(d_vrT[:, :], [128, 16], f32)
                t_bntr = load(d_bntr[:, :], [1, 16], f32)
                t_wfcc = load(d_wfcc[:, :], [16, 1], f16)
                t_bfc = load(d_bfc[:, :], [1, 1], f32)
                t_onesr = load(d_onesr[:, :], [1, 512], f32)
                t_ones32 = load(d_ones32[:, :], [128, 1], f32)
                t_ones32r = load(d_ones32r[:, :], [1, 128], f32)
                t_ident = load(d_ident[:, :], [128, 128], f16)
                t_colsel = load(d_colsel[:, :], [128, P16 * P16], f32)
                t_mask = load(d_mask[:, :], [128, 18], f32)
                t_aidx = load(d_aidx[:, :], [128, DAGROWS // 16], i16)
                t_lidx = load(d_lidx[:, :], [128, DAGROWS // 16], i16)

                for rep in range(nrep):
                    body(nc, tc, tile, mybir, bass_isa, est, locals())

        nc.compile()
        return nc

    # the per-repetition device program body
    def body(nc, tc, tile, mybir, bass_isa, est, L_):
        from contextlib import ExitStack
        AT = mybir.ActivationFunctionType
        AL = mybir.AluOpType
        RG = [list(range(NCORE))]
        t_wlA, t_wlL = L_["t_wlA"], L_["t_wlL"]
        t_bl3, t_ap3 = L_["t_bl3"], L_["t_ap3"]
        t_wpk, t_vlT, t_vrT = L_["t_wpk"], L_["t_vlT"], L_["t_vrT"]
        t_bntr, t_wfcc, t_bfc = L_["t_bntr"], L_["t_wfcc"], L_["t_bfc"]
        t_onesr, t_ident, t_colsel = L_["t_onesr"], L_["t_ident"], L_["t_colsel"]
        t_ones32, t_ones32r = L_["t_ones32"], L_["t_ones32r"]
        t_mask, t_aidx, t_lidx = L_["t_mask"], L_["t_aidx"], L_["t_lidx"]
        d_emb16, d_cc = L_["d_emb16"], L_["d_cc"]
        d_out, d_sbin, d_sbga = L_["d_out"], L_["d_sbin"], L_["d_sbga"]
        d_rsin, d_rsout = L_["d_rsin"], L_["d_rsout"]
        cpool, dagp, hpool, smp = L_["cpool"], L_["dagp"], L_["hpool"], L_["smp"]
        accp, embp, rhsp, segs, hdp = (L_["accp"], L_["embp"], L_["rhsp"],
                                       L_["segs"], L_["hdp"])
        f32 = mybir.dt.float32
        f16 = mybir.dt.float16
        cdt = mybir.dt.float8e4 if use_f8 else mybir.dt.float16

        if KPH == "Z":
            tz = smp.tile([1, 512], f32, tag="z", name="tz")
            nc.vector.memset(tz[:], 0.0)
            nc.sync.dma_start(out=d_out[:, :], in_=tz[:])
            return

        if KPH in ("M", "M1", "M2"):
            # count-matmul machinery in isolation (fake embT)
            PRE_ = int(os.environ.get("KPRE", "6"))
            t_embT = embp.tile([128, SHROWS], f16, tag="embT", name="t_embT")
            nc.vector.memset(t_embT[:], 0.0)
            rts = []
            for pre in range(PRE_):
                rt = rhsp.tile([128, NCHK * 512], cdt, tag="rhs", name="rhs",
                               bufs=PRE_)
                nc.sync.dma_start(out=rt[:], in_=d_cc[pre * 128:(pre + 1) * 128, :])
                rts.append(rt)
            estM = ExitStack()
            ps_seg = estM.enter_context(tc.tile_pool(name="psseg", bufs=3, space="PSUM"))
            sb = None
            for side in range(2):
                for bb in range(NBB):
                    i = side * NBB + bb
                    if KPH in ("M1", "M2") or i < PRE_:
                        rt = rts[i % PRE_]
                    else:
                        rt = rhsp.tile([128, NCHK * 512], cdt, tag="rhs",
                                       name="rhs", bufs=PRE_)
                        nc.sync.dma_start(out=rt[:],
                                          in_=d_cc[i * 128:(i + 1) * 128, :])
                    ps = ps_seg.tile([128, 512], f32, tag="pseg", name="pseg")
                    for chk in range(NCHK):
                        nc.tensor.matmul(ps[:], t_embT[:, chk * 128:(chk + 1) * 128],
                                         rt[:, chk * 512:(chk + 1) * 512],
                                         start=(chk == 0), stop=(chk == NCHK - 1))
                    if KPH == "M2":
                        continue
                    sb = segs.tile([128, 512], f32, tag="segsb", name="sb", bufs=4)
                    nc.vector.tensor_copy(sb[:], ps[:])
                    nc.scalar.dma_start(
                        out=d_rsin[side][bb * 128:(bb + 1) * 128, :], in_=sb[:])
            if KPH == "M2":
                sb = segs.tile([128, 512], f32, tag="segsb", name="sb", bufs=4)
                nc.vector.tensor_copy(sb[:], ps[:])
            estM.close()
            nc.sync.dma_start(out=d_out[:, :], in_=sb[0:1, :])
            return

        # ---------- Phase A: attention logits + softmax partials ----------
        # prefetch big C-count DMAs early (independent of everything)
        PRE = int(os.environ.get("KPRE", "6"))
        rhs_tiles = {}
        if KPH in ("F", "E"):
            for pre in range(PRE):
                side, bb = pre // NBB, pre % NBB
                rt = rhsp.tile([128, NCHK * 512], cdt, tag="rhs", name="rhs", bufs=PRE)
                nc.sync.dma_start(
                    out=rt[:], in_=d_cc[(side * NBB + bb) * 128:(side * NBB + bb + 1) * 128, :])
                rhs_tiles[(side, bb)] = rt

        estA = ExitStack()
        ps_h = estA.enter_context(tc.tile_pool(name="psh", bufs=2, space="PSUM"))
        ps_aw = estA.enter_context(tc.tile_pool(name="psaw", bufs=2, space="PSUM"))
        t_sacc = smp.tile([128, 16], f32, tag="sacc", name="t_sacc")
        nc.vector.memset(t_sacc[:], 0.0)
        aT_all = []
        roff = 0
        for g in range(3):
            vp, Lg = VPAD[g], LS[g]
            co = roff // 16
            n = vp * Lg
            aT = dagp.tile([128, 1, n], f16, tag=f"anc{g}", name="aT", bufs=1)
            nc.gpsimd.dma_gather(
                out_ap=aT[:, :, :n], in_ap=d_emb16[:, :],
                idxs_ap=t_aidx[:, co:co + n // 16],
                num_idxs=n, num_idxs_reg=n, elem_size=H, transpose=True,
                single_packet=False, queue_num=0)
            lT = dagp.tile([128, 1, 5120], f16, tag="leaf", name="lT", bufs=2)
            nc.gpsimd.dma_gather(
                out_ap=lT[:, :, :n], in_ap=d_emb16[:, :],
                idxs_ap=t_lidx[:, co:co + n // 16],
                num_idxs=n, num_idxs_reg=n, elem_size=H, transpose=True,
                single_packet=False, queue_num=0)
            aT_all.append(aT)
            if KPH == "G":
                roff += n
                continue
            for c0 in range(0, vp, 512):
                w = min(512, vp - c0)
                nsub = w // 128
                awp = ps_aw.tile([128, 4, 16], f32, tag="awp", name="awp")
                for l in range(Lg):
                    hp = ps_h.tile([128, 512], f32, tag="hp", name="hp")
                    nc.tensor.matmul(hp[:, :w], t_wlA[:, g * 128:(g + 1) * 128],
                                     aT[:, 0, l * vp + c0:l * vp + c0 + w],
                                     start=True, stop=False)
                    nc.tensor.matmul(hp[:, :w], t_wlL[:, g * 128:(g + 1) * 128],
                                     lT[:, 0, l * vp + c0:l * vp + c0 + w],
                                     start=False, stop=True)
                    hs = hpool.tile([128, 512], f16, tag="hs", name="hs")
                    nc.scalar.activation(hs[:, :w], hp[:, :w], AT.Tanh,
                                         bias=t_bl3[:, g:g + 1])
                    for sub in range(nsub):
                        nc.tensor.matmul(awp[:, sub, l:l + 1],
                                         hs[:, sub * 128:(sub + 1) * 128],
                                         t_ap3[:, g:g + 1],
                                         start=True, stop=True)
                for sub in range(nsub):
                    t = c0 // 128 + sub
                    ex = smp.tile([128, 16], f32, tag="ex", name="ex")
                    nc.scalar.activation(ex[:, :Lg], awp[:, sub, :Lg], AT.Exp)
                    den = smp.tile([128, 1], f32, tag="den", name="den")
                    nc.vector.tensor_reduce(den[:], ex[:, :Lg],
                                            axis=mybir.AxisListType.X, op=AL.add)
                    idn = smp.tile([128, 1], f32, tag="idn", name="idn")
                    nc.vector.reciprocal(idn[:], den[:])
                    sm = smp.tile([128, 16], f32, tag="sm", name="sm")
                    nc.vector.tensor_scalar(out=sm[:, :Lg], in0=ex[:, :Lg],
                                            scalar1=idn[:, 0:1], scalar2=None,
                                            op0=AL.mult)
                    smm = smp.tile([128, 16], f32, tag="smm", name="smm")
                    nc.vector.tensor_scalar(out=smm[:, :Lg], in0=sm[:, :Lg],
                                            scalar1=t_mask[:, MOFF[g] + t:MOFF[g] + t + 1],
                                            scalar2=None, op0=AL.mult)
                    nc.vector.tensor_tensor(
                        out=t_sacc[:, GCOL[g]:GCOL[g] + Lg],
                        in0=t_sacc[:, GCOL[g]:GCOL[g] + Lg],
                        in1=smm[:, :Lg], op=AL.add)
            roff += n
        if KPH == "G":
            estA.close()
            nc.sync.dma_start(out=d_out[0, 0:16], in_=t_sacc[0:1, :])
            return

        # ---------- Phase B: global sbar (PE partition-reduce + AllReduce) ----
        ps_red = estA.enter_context(tc.tile_pool(name="psred", bufs=1, space="PSUM"))
        pr = ps_red.tile([1, 16], f32, tag="pr", name="pr")
        nc.tensor.matmul(pr[:], t_ones32[:, :], t_sacc[:, :], start=True, stop=True)
        t_srow = smp.tile([1, 16], f32, tag="srow", name="t_srow")
        nc.vector.tensor_copy(t_srow[:], pr[:])
        estA.close()
        nc.sync.dma_start(out=d_sbin[:], in_=t_srow[0:1, :])
        nc.gpsimd.collective_compute(
            "AllReduce", AL.add, replica_groups=RG,
            ins=[d_sbin[:]], outs=[d_sbga[:]])
        t_sbr = smp.tile([1, 16], f32, tag="sbr", name="t_sbr")
        nc.sync.dma_start(out=t_sbr[:], in_=d_sbga[:])
        estB = ExitStack()
        ps_bc = estB.enter_context(tc.tile_pool(name="psbc", bufs=1, space="PSUM"))
        pb_ = ps_bc.tile([128, 16], f32, tag="pb_", name="pb_")
        nc.tensor.matmul(pb_[:], t_ones32r[:, :], t_sbr[:, :], start=True, stop=True)
        t_sbb = smp.tile([128, 16], f32, tag="sbb", name="t_sbb")
        nc.vector.tensor_copy(t_sbb[:], pb_[:])
        estB.close()

        if KPH == "A":
            nc.sync.dma_start(out=d_out[0, 0:16], in_=t_sbb[0:1, :])
            return

        # ---------- Phase C: all_emb shard -> [v,h] fp16 lhsT chunks ----------
        estC = ExitStack()
        ps_tr = estC.enter_context(tc.tile_pool(name="pstr", bufs=2, space="PSUM"))
        t_embT = embp.tile([128, SHROWS], f16, tag="embT", name="t_embT")
        for g in range(3):
            vp, Lg = VPAD[g], LS[g]
            aT = aT_all[g]
            eng = nc.vector if g == 0 else nc.gpsimd
            acc = accp.tile([128, 1280], f32, tag=f"acc{g}", name="acc")
            acch = hpool.tile([128, 1280], f16, tag=f"acch{g}", name="acch")
            eng.tensor_scalar(out=acc[:, :vp], in0=aT[:, 0, 0:vp],
                              scalar1=t_sbb[:, GCOL[g]:GCOL[g] + 1],
                              scalar2=None, op0=AL.mult)
            for l in range(1, Lg):
                dst = acc[:, :vp] if l < Lg - 1 else acch[:, :vp]
                eng.scalar_tensor_tensor(
                    out=dst, in0=aT[:, 0, l * vp:(l + 1) * vp],
                    scalar=t_sbb[:, GCOL[g] + l:GCOL[g] + l + 1],
                    in1=acc[:, :vp], op0=AL.mult, op1=AL.add)
            for t in range(NTIL[g]):
                pst = ps_tr.tile([128, 128], f16, tag="pst", name="pst")
                nc.tensor.transpose(pst[:], acch[:, t * 128:(t + 1) * 128],
                                    t_ident[:])
                r0 = GOFF_SH[g] + t * 128
                if t % 2 == 0:
                    nc.vector.tensor_copy(t_embT[:, r0:r0 + 128], pst[:])
                else:
                    nc.scalar.activation(t_embT[:, r0:r0 + 128], pst[:], AT.Copy)
        estC.close()
        if KPH == "C":
            nc.gpsimd.dma_start(out=d_out[0, 0:128], in_=t_embT[0:1, 0:128])
            return

        # ---------- Phase E: count matmuls + per-side ReduceScatter ----------
        estE = ExitStack()
        ps_seg = estE.enter_context(tc.tile_pool(name="psseg", bufs=3, space="PSUM"))
        for side in range(2):
            for bb in range(NBB):
                key = (side, bb)
                if key in rhs_tiles:
                    rt = rhs_tiles.pop(key)
                else:
                    rt = rhsp.tile([128, NCHK * 512], cdt, tag="rhs", name="rhs",
                                   bufs=PRE)
                    nc.sync.dma_start(
                        out=rt[:],
                        in_=d_cc[(side * NBB + bb) * 128:(side * NBB + bb + 1) * 128, :])
                ps = ps_seg.tile([128, 512], f32, tag="pseg", name="pseg")
                for chk in range(NCHK):
                    nc.tensor.matmul(ps[:], t_embT[:, chk * 128:(chk + 1) * 128],
                                     rt[:, chk * 512:(chk + 1) * 512],
                                     start=(chk == 0), stop=(chk == NCHK - 1))
                sb = segs.tile([128, 512], f32, tag="segsb", name="sb", bufs=4)
                nc.vector.tensor_copy(sb[:], ps[:])
                nc.scalar.dma_start(
                    out=d_rsin[side][bb * 128:(bb + 1) * 128, :], in_=sb[:])
            if KPH != "E0":
                nc.gpsimd.collective_compute(
                    "ReduceScatter", AL.add, replica_groups=RG,
                    ins=[d_rsin[side][:, :]], outs=[d_rsout[side][:, :]])
        estE.close()
        if KPH == "E0":
            nc.sync.dma_start(out=d_out[:, :], in_=sb[0:1, :])
            return

        leT = segs.tile([128, 512], f32, tag="leT", name="leT")
        nc.gpsimd.dma_start(out=leT[:], in_=d_rsout[0][:, :])
        reT = segs.tile([128, 512], f32, tag="reT", name="reT")
        nc.gpsimd.dma_start(out=reT[:], in_=d_rsout[1][:, :])
        if KPH == "E":
            nc.sync.dma_start(out=d_out[:, :], in_=leT[0:1, :])
            return

        # ---------- Phase F: NTN head ----------
        estF = ExitStack()
        ps_hd = estF.enter_context(tc.tile_pool(name="pshd", bufs=2, space="PSUM"))
        ps_16 = estF.enter_context(tc.tile_pool(name="ps16", bufs=1, space="PSUM"))
        ps_out = estF.enter_context(tc.tile_pool(name="psout", bufs=1, space="PSUM"))

        # pair_sim pre-activation rows: V@[le;re] + b + bilinear, all in PSUM.
        # The ps16 accumulation group stays open across the interleaved tp
        # matmuls (different PSUM bank) -- hardware-correct, so silence the
        # group check.
        ps16 = ps_16.tile([16, 512], f32, tag="ps16", name="ps16")
        nc.tensor.matmul(ps16[:], t_vlT[:, :], leT[:], start=True, stop=False,
                         skip_group_check=True)
        nc.tensor.matmul(ps16[:], t_vrT[:, :], reT[:], start=False, stop=False,
                         skip_group_check=True)
        nc.tensor.matmul(ps16[:], t_bntr[:, :], t_onesr[:, :], start=False,
                         stop=False, skip_group_check=True)
        for p in range(P16):
            tp = ps_hd.tile([128, 512], f32, tag="tp", name="tp")
            nc.tensor.matmul(tp[:], t_wpk[:, p * 128:(p + 1) * 128], leT[:],
                             start=True, stop=True, skip_group_check=True)
            ml = hdp.tile([128, 512], f32, tag="ml", name="ml", bufs=2)
            nc.vector.tensor_tensor(out=ml[:], in0=tp[:], in1=reT[:], op=AL.mult)
            nc.tensor.matmul(ps16[:], t_colsel[:, p * P16:(p + 1) * P16], ml[:],
                             start=False, stop=(p == P16 - 1),
                             skip_group_check=True)
        th = hdp.tile([16, 512], f16, tag="th", name="th")
        nc.scalar.activation(th[:], ps16[:], AT.Tanh)
        pso = ps_out.tile([1, 512], f32, tag="pso", name="pso")
        nc.tensor.matmul(pso[:], t_wfcc[:, :], th[:], start=True, stop=True)
        sg = hdp.tile([1, 512], f32, tag="sg", name="sg")
        nc.scalar.activation(sg[:], pso[:], AT.Sigmoid, bias=t_bfc[:, 0:1])
        estF.close()
        nc.sync.dma_start(out=d_out[:, :], in_=sg[:])

    nc1 = build(1)
    _trace_kw = {}
    if os.environ.get("KTRACE"):
        _trace_kw = dict(trace=True, tmpdir=os.environ.get("KTRACEDIR") or None)
    res = run_bass_kernel_spmd(nc1, in_maps, list(range(NCORE)), **_trace_kw)
    global LAST_RESULT, LAST_EXEC_NS
    LAST_RESULT = res

    if os.environ.get("KTIME", "1") != "0":
        import time as _time
        try:
            import jax
            from jax.sharding import Mesh, PartitionSpec, NamedSharding
            from jax.experimental.shard_map import shard_map
            import concourse.mybir as mybir2
            from concourse import bass2jax as b2j
            b2j.install_neuronx_cc_hook()
            _conc_cache = {}

            def time_program(nc):
                in_names, out_names, out_avals, zero_outs = [], [], [], []
                pname = nc.partition_id_tensor.name if nc.partition_id_tensor else None
                for alloc in nc.m.functions[0].allocations:
                    if not isinstance(alloc, mybir2.MemoryLocationSet):
                        continue
                    name = alloc.memorylocations[0].name
                    if alloc.kind == "ExternalInput":
                        if name != pname:
                            in_names.append(name)
                    elif alloc.kind == "ExternalOutput":
                        shape = tuple(alloc.tensor_shape)
                        dtype = mybir2.dt.np(alloc.dtype)
                        out_names.append(name)
                        out_avals.append(jax.core.ShapedArray(shape, dtype))
                        zero_outs.append(np.zeros(shape, dtype))
                n_params = len(in_names)
                all_in = list(in_names) + list(out_names)
                if pname is not None:
                    all_in.append(pname)

                def _body(*args):
                    ops = list(args)
                    if pname is not None:
                        ops.append(b2j.partition_id_tensor())
                    return tuple(b2j._bass_exec_p.bind(
                        *ops, out_avals=tuple(out_avals), in_names=tuple(all_in),
                        out_names=tuple(out_names),
                        lowering_input_output_aliases=(),
                        sim_require_finite=True, sim_require_nnan=True, nc=nc))

                devices = jax.devices()[:NCORE]
                mesh = Mesh(np.asarray(devices), ("core",))
                nio = n_params + len(out_names)
                fn = jax.jit(shard_map(_body, mesh=mesh,
                                       in_specs=(PartitionSpec("core"),) * nio,
                                       out_specs=(PartitionSpec("core"),) * len(out_names),
                                       check_rep=False),
                             donate_argnums=tuple(range(n_params, nio)),
                             keep_unused=True)
                sh = NamedSharding(mesh, PartitionSpec("core"))
                ckey = tuple(in_names)
                if ckey not in _conc_cache:
                    _conc_cache[ckey] = [jax.device_put(np.concatenate(
                        [np.asarray(in_maps[c][n]) for c in range(NCORE)], axis=0), sh)
                        for n in in_names]
                conc = _conc_cache[ckey]
                NIT = int(os.environ.get("KITER", "6"))
                BURST = int(os.environ.get("KBURST", "32"))
                best = None
                _times = []
                for it in range(NIT):
                    zss = [[jax.device_put(
                              np.zeros((NCORE * z.shape[0], *z.shape[1:]), z.dtype), sh)
                            for z in zero_outs] for _ in range(BURST)]
                    jax.block_until_ready(fn(*conc, *zss[0]))
                    t0 = _time.perf_counter()
                    outs = [fn(*conc, *zs) for zs in zss[1:]]
                    jax.block_until_ready(outs)
                    dt = (_time.perf_counter() - t0) / max(1, BURST - 1)
                    _times.append(dt)
                    if it > 0:
                        best = dt if best is None else min(best, dt)
                if os.environ.get("KVERBOSE"):
                    print("per-call times (ms):", [round(t * 1e3, 3) for t in _times])
                return best

            t1 = time_program(nc1)
            R = int(os.environ.get("KREP", "8"))
            if R > 1:
                ncR = build(R)
                tR = time_program(ncR)
                exec_s = max((tR - t1) / (R - 1), 1e-9)
                if os.environ.get("KVERBOSE"):
                    print(f"t1={t1*1e3:.3f}ms tR={tR*1e3:.3f}ms "
                          f"-> per-exec {(tR-t1)/(R-1)*1e3:.3f}ms")
            else:
                exec_s = t1
            LAST_EXEC_NS = int(exec_s * 1e9)
        except Exception as e:
            import traceback
            traceback.print_exc()
            print("KTIME path failed:", repr(e))
    outs = [np.asarray(res.results[c]["out"]).reshape(BLOC) for c in range(NCORE)]
    return np.concatenate(outs).astype(np.float32)


if __name__ == "__main__":
    pass
